# Initial kernel scaffold
#
"""CMHSA Trainium2 kernel: cross-head-mixed attention with instance norm.

Math (per batch element b, all on one core; B=8 -> 8 cores data-parallel):
  xf [C=256, T=1024]
  q = Wq@xf, k = Wk@xf, v = Wv@xf            (C x T)
  s[h] = sum_g w_head[h,g] * (q_g^T k_g) * sc    -- folded: qt_h = q scaled
         rowwise by w_head[h, c//32]; s^T[h] = k^T @ qt_h  (K=256 contraction)
  E = exp(sc * s^T)  [t, qp] layout
  r[qp] = sum_t E (ones row in AV lhsT), rs2[qp] = sum_t E^2 (ones matmul)
  Zraw^T[d, qp] = sum_t v^T[t,d] E[t,qp]     (AV matmul, lhsT = [v^T | 1])
  a = E/r; mean = 1/T exact; var = (sum_qp rs2/r^2)/T^2 - 1/T^2
  alpha = gamma*rsqrt(var+eps); Zs = Zraw * (alpha/r)
  out = alpha*(Zraw/r) + (beta - alpha/T)*vsum  -> projection by Wp with the
  torch raw-view head scramble folded into Wp^T row slicing:
    y_pre[t=128h+m, c=32j+d] = Z_h[q=8m+j, d]
    y^T[o, t] = sum_j Wp^T[32j:32j+32, o] @ Zs_h[:, j::8]  (+ bias fixups)
"""

import math
import os
import warnings

warnings.filterwarnings("ignore")

import numpy as np

import concourse.bass as bass
import concourse.mybir as mybir
import concourse.tile as tile
from concourse import library_config
from concourse.bass_utils import run_bass_kernel_spmd

B, C, T, NH, HD, P = 8, 256, 1024, 8, 32, 128
EPS = 1e-5
SCALE = 1.0 / math.sqrt(HD)
F32 = mybir.dt.float32
F32R = mybir.dt.float32r
BF16 = mybir.dt.bfloat16
FP8 = mybir.dt.float8e5
AF = mybir.ActivationFunctionType
ALU = mybir.AluOpType
N_CORES = 8


def _r(ap):
    return ap.bitcast(F32R)


def _split_excess_waits(nc, max_waits=1):
    """This walrus build rejects >2 sem-waits on one instruction
    ("Too many sync wait commands" in setupSyncWait). Hoist excess waits
    onto same-engine NoOps inserted right before the offending instruction."""
    for f in nc.m.functions:
        for blk in f.blocks:
            insts = list(blk.instructions)
            out, changed = [], False
            for inst in insts:
                si = inst.sync_info
                waits = list(si.on_wait) if si and si.on_wait else []
                if len(waits) > max_waits:
                    extra, keep = waits[:-max_waits], waits[-max_waits:]
                    for w in extra:
                        nop = mybir.InstNoOp(
                            name=f"I-sw-{nc.next_id()}",
                            ins=[],
                            outs=[],
                            sync_info=mybir.SyncInfo(on_wait=[w], on_update=[]),
                            engine=inst.engine,
                        )
                        nc.register_instruction(nop)
                        out.append(nop)
                    inst.sync_info = mybir.SyncInfo(
                        on_wait=keep, on_update=list(si.on_update or [])
                    )
                    changed = True
                out.append(inst)
            if changed:
                blk.instructions = out


def build_bass(reps=1):
    nc = bass.Bass("TRN2", target_bir_lowering=False, debug=False)

    xf_d = nc.dram_tensor("xf", [C, T], F32R, kind="ExternalInput")
    wqt_d = nc.dram_tensor("wqt", [C, C], F32R, kind="ExternalInput")
    wkt_d = nc.dram_tensor("wkt", [C, C], F32R, kind="ExternalInput")
    wvt_d = nc.dram_tensor("wvt", [C, C], F32R, kind="ExternalInput")
    wst_d = nc.dram_tensor("wst", [C, NH], F32, kind="ExternalInput")
    wpt_d = nc.dram_tensor("wpt", [HD, 8, C], F32R, kind="ExternalInput")
    wpct_d = nc.dram_tensor("wpct", [HD, C], F32R, kind="ExternalInput")
    bp_d = nc.dram_tensor("bp", [C, 1], F32, kind="ExternalInput")
    gamma_d = nc.dram_tensor("gamma", [NH, 1], F32, kind="ExternalInput")
    beta_d = nc.dram_tensor("beta", [NH, 1], F32, kind="ExternalInput")
    yt_d = nc.dram_tensor("yt", [C, T], F32, kind="ExternalOutput")

    with tile.TileContext(nc) as tc:
        with (
            tc.tile_pool(name="w", bufs=1) as wp,
            tc.tile_pool(name="stream", bufs=2) as sp,
        ):
            # ---- persistent SBUF tensors ----
            xf_sb = [wp.tile([P, T], F32R, name=f"xf{i}", tag=f"xf{i}") for i in range(2)]
            wqt_sb = [wp.tile([P, C], F32R, name=f"wqt{i}", tag=f"wqt{i}") for i in range(2)]
            wkt_sb = [wp.tile([P, C], F32R, name=f"wkt{i}", tag=f"wkt{i}") for i in range(2)]
            wvt_sb = [wp.tile([P, C], F32R, name=f"wvt{i}", tag=f"wvt{i}") for i in range(2)]
            wst_sb = [wp.tile([P, NH], F32, name=f"wst{i}", tag=f"wst{i}") for i in range(2)]
            wpt_sb = wp.tile([HD, 8, C], F32R, name="wptj", tag="wptj")
            wpct_sb = wp.tile([HD, C], F32R, name="wpct", tag="wpct")
            bp_sb = [wp.tile([P, 1], F32, name=f"bp{i}", tag=f"bp{i}") for i in range(2)]
            gamma_sb = wp.tile([NH, 1], F32, name="gamma", tag="gamma")
            beta_sb = wp.tile([NH, 1], F32, name="beta", tag="beta")
            q_sb = [wp.tile([P, T], F32R, name=f"q{i}", tag=f"q{i}") for i in range(2)]
            k_sb = [wp.tile([P, T], F32R, name=f"k{i}", tag=f"k{i}") for i in range(2)]
            # v^T tiles: per t-chunk tm, 8 heads x (32 cols + ones col)
            vt_sb = wp.tile([P, 8, NH * 33], F32R, name="vt", tag="vt")
            ones_col = wp.tile([P, 1], BF16, name="ones", tag="ones")
            # Z rows 0..31, r row 32, rs2 row 64 per head block of 1024 qp
            zr_sb = wp.tile([65, NH * T], F32R, name="zr", tag="zr")
            vs_row = wp.tile([1, C], F32R, name="vsrow", tag="vsrow")
            vs_dh = wp.tile([HD, NH], F32R, name="vsdh", tag="vsdh")
            y_sb = [wp.tile([P, T], F32, name=f"y{i}", tag=f"y{i}") for i in range(2)]
            rstat = wp.tile([NH, T], F32, name="rstat", tag="rstat")
            rs2h = wp.tile([NH, T], F32, name="rs2h", tag="rs2h")
            rinv = wp.tile([NH, T], F32, name="rinv", tag="rinv")
            ss = wp.tile([NH, 1], F32, name="ss", tag="ss")
            vpe = wp.tile([NH, 1], F32, name="vpe", tag="vpe")
            sdv = wp.tile([NH, 1], F32, name="sdv", tag="sdv")
            alpha = wp.tile([NH, 1], F32, name="alpha", tag="alpha")
            biash = wp.tile([NH, 1], F32R, name="biash", tag="biash")
            biasrow = wp.tile([1, NH], F32R, name="biasrow", tag="biasrow")
            u_sb = [wp.tile([P, NH], F32, name=f"u{i}", tag=f"u{i}") for i in range(2)]
            ones_colr = wp.tile([P, 1], F32R, name="ones_colr", tag="ones_colr")
            onesr32 = wp.tile([1, 32], F32R, name="onesr32", tag="onesr32")
            onesr128 = wp.tile([1, P], F32R, name="onesr128", tag="onesr128")
            onesf128 = wp.tile([1, P], F32, name="onesf128", tag="onesf128")
            s1r = wp.tile([NH, T], F32R, name="s1r", tag="s1r")
            s1all = wp.tile([1, NH * T], F32R, name="s1all", tag="s1all")

            def _one_rep():
                # ---- phase 0: loads ----
                for i in range(2):
                    nc.sync.dma_start(xf_sb[i][:], xf_d[i * P : (i + 1) * P, :])
                    nc.sync.dma_start(wqt_sb[i][:], wqt_d[i * P : (i + 1) * P, :])
                    nc.sync.dma_start(wkt_sb[i][:], wkt_d[i * P : (i + 1) * P, :])
                    nc.sync.dma_start(wvt_sb[i][:], wvt_d[i * P : (i + 1) * P, :])
                    nc.sync.dma_start(wst_sb[i][:], wst_d[i * P : (i + 1) * P, :])
                    nc.sync.dma_start(bp_sb[i][:], bp_d[i * P : (i + 1) * P, :])
                nc.sync.dma_start(wpt_sb[:], wpt_d[:])
                nc.sync.dma_start(wpct_sb[:], wpct_d[:])
                nc.sync.dma_start(gamma_sb[:], gamma_d[:])
                nc.sync.dma_start(beta_sb[:], beta_d[:])
                nc.vector.memset(ones_col[:], 1.0)
                nc.vector.memset(onesf128[:], 1.0)
                nc.vector.tensor_copy(ones_colr[:], ones_col[:])
                nc.vector.tensor_copy(onesr32[:], onesf128[:, 0:32])
                nc.vector.tensor_copy(onesr128[:], onesf128[:])
                # ones columns inside vt (lhsT column 32 of each head block)
                for tm8 in range(8):
                    vt3 = vt_sb[:, tm8, :].rearrange("p (h e) -> p h e", e=33)
                    nc.vector.tensor_copy(
                        vt3[:, :, 32:33], ones_colr[:, 0:1].broadcast_to((P, 8, 1))
                    )

                # ---- phase 1: projections ----
                with tc.tile_pool(name="psA", bufs=2, space=bass.MemorySpace.PSUM) as psA:
                    for wt, dst in ((wqt_sb, q_sb), (wkt_sb, k_sb)):
                        for co in range(2):
                            for tn in range(2):
                                pq = psA.tile([P, 512], F32, name="qk", tag="qk")
                                for kc in range(2):
                                    nc.tensor.matmul(
                                        pq[:],
                                        wt[kc][:, co * P : (co + 1) * P],
                                        xf_sb[kc][:, tn * 512 : (tn + 1) * 512],
                                        start=(kc == 0),
                                        stop=(kc == 1),
                                    )
                                nc.scalar.activation(
                                    dst[co][:, tn * 512 : (tn + 1) * 512], pq[:], AF.Copy
                                )
                    # v^T = xf^T @ Wv^T, written per t-chunk with head-stride 33
                    pvs = psA.tile([1, C], F32, name="vs", tag="vs")
                    for tm in range(8):
                        pv = psA.tile([P, C], F32, name="vt", tag="vt")
                        for kc in range(2):
                            nc.tensor.matmul(
                                pv[:],
                                xf_sb[kc][:, tm * P : (tm + 1) * P],
                                wvt_sb[kc][:],
                                start=(kc == 0),
                                stop=(kc == 1),
                            )
                        src = pv[:].rearrange("p (h d) -> p h d", h=NH)
                        dst3 = vt_sb[:, tm, :].rearrange("p (h e) -> p h e", e=33)
                        nc.scalar.activation(dst3[:, :, 0:32], src[:], AF.Copy)
                        nc.tensor.matmul(
                            pvs[:],
                            ones_colr[:],
                            dst3[:, :, 0:32].rearrange("p h d -> p d h"),
                            start=(tm == 0),
                            stop=(tm == 7),
                        )
                    nc.scalar.activation(vs_row[:], pvs[:], AF.Copy)

                # ---- phase 2: streaming attention ----
                with (
                    tc.tile_pool(name="psS", bufs=2, space=bass.MemorySpace.PSUM) as psS,
                    tc.tile_pool(name="psAV", bufs=2, space=bass.MemorySpace.PSUM) as psAV,
                ):
                    for h in range(NH):
                        qt = sp.tile([P, 2, T], F32R, name="qt", tag="qt")
                        for kc in range(2):
                            nc.vector.tensor_scalar_mul(
                                qt[:, kc, :], q_sb[kc][:], wst_sb[kc][:, h : h + 1]
                            )
                        pav = psAV.tile([65, T], F32, name="av", tag="av")
                        for tm in range(8):
                            ps = psS.tile([P, T], F32, name="s", tag="s")
                            for kc in range(2):
                                for qh in range(2):
                                    nc.tensor.matmul(
                                        ps[:, qh * 512 : (qh + 1) * 512],
                                        k_sb[kc][:, tm * P : (tm + 1) * P],
                                        qt[:, kc, qh * 512 : (qh + 1) * 512],
                                        start=(kc == 0),
                                        stop=(kc == 1),
                                    )
                            et = sp.tile([P, T], F32R, name="E", tag="E", bufs=3)
                            nc.scalar.activation(et[:], ps[:], AF.Exp, scale=SCALE)
                            sq = sp.tile([P, T], BF16, name="SQ", tag="SQ", bufs=3)
                            nc.vector.tensor_mul(sq[:], et[:], et[:])
                            for qh in range(2):
                                nc.tensor.matmul(
                                    pav[0:33, qh * 512 : (qh + 1) * 512],
                                    vt_sb[:, tm, 33 * h : 33 * h + 33],
                                    et[:, qh * 512 : (qh + 1) * 512],
                                    start=(tm == 0),
                                    stop=(tm == 7),
                                    skip_group_check=True,
                                )
                            for qh in range(2):
                                nc.tensor.matmul(
                                    pav[64:65, qh * 512 : (qh + 1) * 512],
                                    ones_col[:],
                                    sq[:, qh * 512 : (qh + 1) * 512],
                                    start=(tm == 0),
                                    stop=(tm == 7),
                                    skip_group_check=True,
                                )
                        nc.scalar.activation(
                            zr_sb[0:33, h * T : (h + 1) * T], pav[0:33, :], AF.Copy
                        )
                        nc.scalar.activation(
                            zr_sb[64:65, h * T : (h + 1) * T], pav[64:65, :], AF.Copy
                        )
                        nc.sync.dma_start(
                            rstat[h : h + 1, :],
                            zr_sb[32:33, h * T : (h + 1) * T].bitcast(F32),
                        )
                        nc.sync.dma_start(
                            rs2h[h : h + 1, :],
                            zr_sb[64:65, h * T : (h + 1) * T].bitcast(F32),
                        )

                # ---- phase 3: instance-norm stats + Z scaling ----
                nc.vector.reciprocal(rinv[:], rstat[:])
                nc.vector.tensor_mul(rs2h[:], rs2h[:], rinv[:])
                nc.vector.scalar_tensor_tensor(
                    rs2h[:], rs2h[:], 1.0, rinv[:],
                    op0=ALU.mult, op1=ALU.mult, accum_out=ss[:],
                )
                nc.vector.tensor_scalar(
                    vpe[:], ss[:], 1.0 / (T * T), EPS - 1.0 / (T * T),
                    op0=ALU.mult, op1=ALU.add,
                )
                nc.scalar.activation(sdv[:], vpe[:], AF.Sqrt)
                nc.vector.reciprocal(alpha[:], sdv[:])
                nc.vector.tensor_mul(alpha[:], alpha[:], gamma_sb[:])
                nc.vector.scalar_tensor_tensor(
                    biash[:], alpha[:], -1.0 / T, beta_sb[:], op0=ALU.mult, op1=ALU.add
                )
                # s1 = alpha / r
                nc.vector.tensor_scalar_mul(s1r[:], rinv[:], alpha[:, 0:1])

                # ---- phase 4: Z scaling + vsum fixup vector ----
                with tc.tile_pool(name="psB", bufs=2, space=bass.MemorySpace.PSUM) as psB:
                    nc.sync.dma_start(s1all[:], s1r[:])
                    for h in range(NH):
                        pb = psB.tile([32, T], F32, name="pb", tag="pb", bufs=2)
                        for nhf in range(2):
                            nc.tensor.matmul(
                                pb[:, nhf * 512 : (nhf + 1) * 512],
                                onesr32[:],
                                s1all[0:1, h * T + nhf * 512 : h * T + (nhf + 1) * 512],
                                start=True,
                                stop=True,
                            )
                        nc.vector.tensor_mul(
                            zr_sb[0:32, h * T : (h + 1) * T],
                            zr_sb[0:32, h * T : (h + 1) * T],
                            pb[:],
                        )
                    nc.sync.dma_start(vs_dh[:], vs_row[:])
                    nc.sync.dma_start(biasrow[:], biash[:])
                    pbb = psB.tile([P, NH], F32, name="pbb", tag="ub", bufs=2)
                    nc.tensor.matmul(
                        pbb[:], onesr128[:], biasrow[:], start=True, stop=True
                    )
                    for oc in range(2):
                        pu = psB.tile([P, NH], F32, name="u", tag="ub", bufs=2)
                        nc.tensor.matmul(
                            pu[:],
                            wpct_sb[:, oc * P : (oc + 1) * P],
                            vs_dh[:],
                            start=True,
                            stop=True,
                        )
                        nc.scalar.activation(u_sb[oc][:], pu[:], AF.Copy)
                        nc.vector.tensor_mul(u_sb[oc][:], u_sb[oc][:], pbb[:])
                        nc.vector.tensor_scalar_add(
                            u_sb[oc][:], u_sb[oc][:], bp_sb[oc][:, 0:1]
                        )

                    # ---- phase 5: projection with head-scramble folded in ----
                    zrr = zr_sb[0:32, :].rearrange(
                        "p (h m j) -> p h m j", h=NH, m=P, j=8
                    )
                    for oc in range(2):
                        py = psB.tile([P, T], F32, name="y", tag="y", bufs=1)
                        for j in range(8):
                            lhsT = wpt_sb[:, j, oc * P : (oc + 1) * P]
                            for hf in range(2):
                                nc.tensor.matmul(
                                    py[:, hf * 512 : (hf + 1) * 512],
                                    lhsT,
                                    zrr[:, 4 * hf : 4 * hf + 4, :, j],
                                    start=(j == 0),
                                    stop=(j == 7),
                                )
                        yv = y_sb[oc][:].rearrange("p (h m) -> p h m", h=NH)
                        pyv = py[:].rearrange("p (h m) -> p h m", h=NH)
                        bias_b = u_sb[oc][:, :, None].broadcast_to((P, NH, P))
                        nc.vector.tensor_add(yv, pyv, bias_b)
                        nc.sync.dma_start(yt_d[oc * P : (oc + 1) * P, :], y_sb[oc][:])

            for _rep in range(reps):
                _one_rep()

    _split_excess_waits(nc)
    return nc


def _host_inputs(x, Wq, Wk, Wv, w_head, gamma, beta, Wp, bp):
    f = np.float32
    common = {
        "wqt": np.ascontiguousarray(np.asarray(Wq, f).T),
        "wkt": np.ascontiguousarray(np.asarray(Wk, f).T),
        "wvt": np.ascontiguousarray(np.asarray(Wv, f).T),
        "wst": np.ascontiguousarray(np.repeat(np.asarray(w_head, f), HD, axis=1).T),
        "wpt": np.ascontiguousarray(
            np.asarray(Wp, f).T.reshape(8, HD, C).transpose(1, 0, 2)
        ),
        "wpct": np.ascontiguousarray(
            np.asarray(Wp, f).T.reshape(8, HD, C).sum(0)
        ),
        "bp": np.ascontiguousarray(np.asarray(bp, f).reshape(C, 1)),
        "gamma": np.ascontiguousarray(np.asarray(gamma, f).reshape(NH, 1)),
        "beta": np.ascontiguousarray(np.asarray(beta, f).reshape(NH, 1)),
    }
    xs = np.asarray(x, f).reshape(B, C, T)
    return [
        {"xf": np.ascontiguousarray(xs[b]), **common} for b in range(B)
    ]


_NC_CACHE = {}


def _get_nc(reps=1):
    if reps not in _NC_CACHE:
        _NC_CACHE[reps] = build_bass(reps=reps)
    return _NC_CACHE[reps]


def run(inputs, trace=False):
    nc = _get_nc()
    in_maps = _host_inputs(**inputs)
    res = run_bass_kernel_spmd(
        nc, in_maps, core_ids=list(range(N_CORES)), trace=trace
    )
    y = np.stack([res.results[b]["yt"] for b in range(B)], axis=0)
    return y.reshape(B, C, 32, 32).astype(np.float32), res


def _build_sharded(reps=1):
    """Replicate bass2jax.run_bass_via_pjrt but return a reusable callable
    (no donation) so device execution can be timed over many iterations."""
    import jax
    from jax.sharding import Mesh, PartitionSpec
    from jax.experimental.shard_map import shard_map
    from concourse import bass2jax

    nc = _get_nc(reps)
    bass2jax.install_neuronx_cc_hook()
    part_name = nc.partition_id_tensor.name if nc.partition_id_tensor else None
    in_names, out_names, out_avals = [], [], []
    for alloc in nc.m.functions[0].allocations:
        if not isinstance(alloc, mybir.MemoryLocationSet):
            continue
        name = alloc.memorylocations[0].name
        if alloc.kind == "ExternalInput":
            if name == part_name:
                continue
            in_names.append(name)
        elif alloc.kind == "ExternalOutput":
            out_names.append(name)
            out_avals.append(
                jax.core.ShapedArray(
                    tuple(alloc.tensor_shape), mybir.dt.np(alloc.dtype)
                )
            )
    n_params = len(in_names)
    all_in = in_names + out_names
    if part_name is not None:
        all_in = all_in + [part_name]

    def _body(*args):
        operands = list(args)
        if part_name is not None:
            operands.append(bass2jax.partition_id_tensor())
        outs = bass2jax._bass_exec_p.bind(
            *operands,
            out_avals=tuple(out_avals),
            in_names=tuple(all_in),
            out_names=tuple(out_names),
            lowering_input_output_aliases=(),
            sim_require_finite=True,
            sim_require_nnan=True,
            nc=nc,
        )
        return tuple(outs)

    devices = jax.devices()[:N_CORES]
    mesh = Mesh(np.asarray(devices), ("core",))
    nouts = len(out_names)
    sharded = jax.jit(
        shard_map(
            _body,
            mesh=mesh,
            in_specs=(PartitionSpec("core"),) * (n_params + nouts),
            out_specs=(PartitionSpec("core"),) * nouts,
            check_rep=False,
        ),
        keep_unused=True,
    )
    return sharded, mesh, in_names, out_names, out_avals


def timed_run(inputs, iters=20, reps=1):
    import time
    import jax
    from jax.sharding import NamedSharding, PartitionSpec

    sharded, mesh, in_names, out_names, out_avals = _build_sharded(reps)
    in_maps = _host_inputs(**inputs)
    sh = NamedSharding(mesh, PartitionSpec("core"))
    dev_in = [
        jax.device_put(
            np.concatenate([in_maps[c][n] for c in range(N_CORES)], axis=0), sh
        )
        for n in in_names
    ]
    dev_zero = [
        jax.device_put(
            np.zeros((N_CORES * a.shape[0], *a.shape[1:]), a.dtype), sh
        )
        for a in out_avals
    ]
    out = sharded(*dev_in, *dev_zero)
    jax.block_until_ready(out)
    # blocking per-call (includes full dispatch round trip)
    times = []
    for _ in range(max(3, iters // 4)):
        t0 = time.perf_counter()
        out = sharded(*dev_in, *dev_zero)
        jax.block_until_ready(out)
        times.append(time.perf_counter() - t0)
    # pipelined: submit all, block once -> amortizes host/axon dispatch
    t0 = time.perf_counter()
    outs = [sharded(*dev_in, *dev_zero) for _ in range(iters)]
    jax.block_until_ready(outs)
    pipelined = (time.perf_counter() - t0) / iters
    times.append(pipelined)
    print(f"pipelined per-call: {pipelined * 1e9:.0f} ns")
    y = np.asarray(outs[-1][out_names.index("yt")]).reshape(N_CORES, C, T)
    return y.reshape(B, C, 32, 32).astype(np.float32), times


def kernel(**inputs):
    y, _ = run(inputs, trace=False)
    return y


def numpy_check():
    """CoreSim single-core check against a numpy reference (core 0 data)."""
    from concourse.bass_interp import CoreSim

    rng = np.random.default_rng(0)
    x = rng.standard_normal((B, C, 32, 32), np.float32)
    Wq = (rng.standard_normal((C, C)) * 0.05).astype(np.float32)
    Wk = (rng.standard_normal((C, C)) * 0.05).astype(np.float32)
    Wv = (rng.standard_normal((C, C)) * 0.05).astype(np.float32)
    w_head = (rng.standard_normal((NH, NH)) * 0.3).astype(np.float32)
    gamma = rng.uniform(0.5, 1.5, NH).astype(np.float32)
    beta = (rng.standard_normal(NH) * 0.1).astype(np.float32)
    Wp = (rng.standard_normal((C, C)) * 0.05).astype(np.float32)
    bp = (rng.standard_normal(C) * 0.05).astype(np.float32)
    inputs = dict(
        x=x, Wq=Wq, Wk=Wk, Wv=Wv, w_head=w_head, gamma=gamma, beta=beta,
        Wp=Wp, bp=bp,
    )

    def ref_np(x, Wq, Wk, Wv, w_head, gamma, beta, Wp, bp):
        Bn, Cn, H, W = x.shape
        Tn = H * W
        hd = Cn // NH
        sc = float(hd) ** -0.5
        xf = x.reshape(Bn, Cn, Tn).astype(np.float64)
        q = np.einsum("oc,bct->bot", Wq, xf).reshape(Bn, NH, hd, Tn)
        k = np.einsum("oc,bct->bot", Wk, xf).reshape(Bn, NH, hd, Tn)
        v = np.einsum("oc,bct->bot", Wv, xf).reshape(Bn, NH, hd, Tn)
        s = np.einsum("bhdq,bhdt->bhqt", q, k) * sc
        s = np.einsum("hg,bgqt->bhqt", w_head.astype(np.float64), s)
        s = s - s.max(axis=-1, keepdims=True)
        e = np.exp(s)
        a = e / e.sum(-1, keepdims=True)
        mean = a.mean(axis=(2, 3), keepdims=True)
        var = a.var(axis=(2, 3), keepdims=True)
        g = gamma.astype(np.float64)[None, :, None, None]
        bt = beta.astype(np.float64)[None, :, None, None]
        a = (a - mean) / np.sqrt(var + EPS) * g + bt
        out = np.einsum("bhqt,bhdt->bhqd", a, v)
        y = out.reshape(Bn, Tn, Cn)
        y = np.einsum("btc,oc->bto", y, Wp.astype(np.float64)) + bp
        return y.transpose(0, 2, 1).reshape(Bn, Cn, H, W)

    expected = ref_np(**inputs)[0]  # core 0

    nc = _get_nc()
    in_maps = _host_inputs(**inputs)
    sim = CoreSim(nc, trace=False)
    for name, arr in in_maps[0].items():
        sim.tensor(name)[:] = arr
    sim.simulate(check_with_hw=False)
    got = np.array(sim.tensor("yt")).reshape(C, 32, 32)
    err = np.abs(got - expected) / (np.abs(expected) + 1e-3)
    print("max rel err (sim vs numpy f64):", err.max())
    print("mean rel err:", err.mean())
    return err.max()


if __name__ == "__main__":
    numpy_check()



# revision 5
# speedup vs baseline: 1.2839x; 1.2839x over previous
"""CMHSA Trainium2 kernel v2: per-head fused tail, direct-PSUM stats.

Per core (B=8 -> 8 cores data-parallel):
  q = Wq@xf, k = Wk@xf, v = Wv@xf            (C x T)
  s^T[h] = k^T @ (q . wst_h)  (K=256, w_head folded into q scaling via ACT
           Copy with per-partition scale)
  E = exp(sc * s^T) bf16 [t, q];  sq = E*E bf16 (DVE 2x)
  pav rows: 0:32 Z = v^T E (lhsT = [v^T | 1] -> row 32 = r), 64 rs2 = 1^T sq
  per-head stats read pav rows straight from PSUM (32-aligned partition
  shifts are legal): rinv = 1/r; ss = sum_q rs2*rinv^2; var = (ss-1)/T^2;
  alpha = gamma * rsqrt(var+eps)  (Newton rsqrt on DVE, no ACT table switch)
  s1 = alpha*rinv -> s1row (bf16, partition 0); biash = beta - alpha/T
  tail: gpsimd bcast s1 -> [32,T]; zs = zrb*s1 (bf16 2x);
    y[:, 128h:128h+128] = sum_j wpt_j^T @ zs[:, j::8] + u[:, h]; DMA out
  u[:, oc, h] = wtld[:, oc, h]*biash_h + bp   (wtld = wpct^T vs_dh, phase 1)
  Stats for head h are spread over head h+1's tm slots; tail of h runs
  during head h+2. The DVE/ACT/Pool queues stay off the PE critical path.
"""

import math
import os
import warnings

warnings.filterwarnings("ignore")

import numpy as np
import ml_dtypes

import concourse.bass as bass
import concourse.mybir as mybir
import concourse.tile as tile
from concourse import library_config
from concourse.bass_utils import run_bass_kernel_spmd

B, C, T, NH, HD, P = 8, 256, 1024, 8, 32, 128
EPS = 1e-5
SCALE = 1.0 / math.sqrt(HD)
F32 = mybir.dt.float32
F32R = mybir.dt.float32r
BF16 = mybir.dt.bfloat16
I32 = mybir.dt.int32
AF = mybir.ActivationFunctionType
ALU = mybir.AluOpType
N_CORES = 8
MAGIC = 0x5F3759DF
KVAR = set(os.environ.get("KVAR", "").split(","))


def _split_excess_waits(nc, max_waits=1):
    """This walrus build rejects >2 sem-waits on one instruction
    ("Too many sync wait commands" in setupSyncWait). Hoist excess waits
    onto same-engine NoOps inserted right before the offending instruction."""
    for f in nc.m.functions:
        for blk in f.blocks:
            insts = list(blk.instructions)
            out, changed = [], False
            for inst in insts:
                si = inst.sync_info
                waits = list(si.on_wait) if si and si.on_wait else []
                if len(waits) > max_waits:
                    extra, keep = waits[:-max_waits], waits[-max_waits:]
                    for w in extra:
                        nop = mybir.InstNoOp(
                            name=f"I-sw-{nc.next_id()}",
                            ins=[],
                            outs=[],
                            sync_info=mybir.SyncInfo(on_wait=[w], on_update=[]),
                            engine=inst.engine,
                        )
                        nc.register_instruction(nop)
                        out.append(nop)
                    inst.sync_info = mybir.SyncInfo(
                        on_wait=keep, on_update=list(si.on_update or [])
                    )
                    changed = True
                out.append(inst)
            if changed:
                blk.instructions = out


def build_bass(reps=1):
    nc = bass.Bass("TRN2", target_bir_lowering=False, debug=False)

    xf_d = nc.dram_tensor("xf", [C, T], F32R, kind="ExternalInput")
    wqt_d = nc.dram_tensor("wqt", [C, C], F32R, kind="ExternalInput")
    wkt_d = nc.dram_tensor("wkt", [C, C], F32R, kind="ExternalInput")
    wvt_d = nc.dram_tensor("wvt", [C, C], F32R, kind="ExternalInput")
    wst_d = nc.dram_tensor("wst", [C, NH], F32, kind="ExternalInput")
    wpt_d = nc.dram_tensor("wpt", [HD, 8, C], BF16, kind="ExternalInput")
    wpct_d = nc.dram_tensor("wpct", [HD, C], F32R, kind="ExternalInput")
    bp_d = nc.dram_tensor("bp", [C, 1], F32, kind="ExternalInput")
    gamma_d = nc.dram_tensor("gamma", [1, NH], F32, kind="ExternalInput")
    beta_d = nc.dram_tensor("beta", [1, NH], F32, kind="ExternalInput")
    yt_d = nc.dram_tensor("yt", [C, T], F32, kind="ExternalOutput")

    with tile.TileContext(nc) as tc:
        with (
            tc.tile_pool(name="w", bufs=1) as wp,
            tc.tile_pool(name="stream", bufs=2) as sp,
        ):
            # ---- persistent SBUF tensors ----
            xf_sb = [wp.tile([P, T], F32R, name=f"xf{i}", tag=f"xf{i}") for i in range(2)]
            wqt_sb = [wp.tile([P, C], F32R, name=f"wqt{i}", tag=f"wqt{i}") for i in range(2)]
            wkt_sb = [wp.tile([P, C], F32R, name=f"wkt{i}", tag=f"wkt{i}") for i in range(2)]
            wvt_sb = [wp.tile([P, C], F32R, name=f"wvt{i}", tag=f"wvt{i}") for i in range(2)]
            wst_sb = [wp.tile([P, NH], F32, name=f"wst{i}", tag=f"wst{i}") for i in range(2)]
            wpt_sb = wp.tile([HD, 8, C], BF16, name="wptj", tag="wptj")
            wpct_sb = wp.tile([HD, C], F32R, name="wpct", tag="wpct")
            bp_sb = [wp.tile([P, 1], F32, name=f"bp{i}", tag=f"bp{i}") for i in range(2)]
            gamma_sb = wp.tile([1, NH], F32, name="gamma", tag="gamma")
            beta_sb = wp.tile([1, NH], F32, name="beta", tag="beta")
            q_sb = [wp.tile([P, T], F32R, name=f"q{i}", tag=f"q{i}") for i in range(2)]
            k_sb = [wp.tile([P, T], F32R, name=f"k{i}", tag=f"k{i}") for i in range(2)]
            # v^T tiles: per t-chunk tm, 8 heads x (32 cols + ones col), bf16
            vt_sb = wp.tile([P, 8, NH * 33], BF16, name="vt", tag="vt")
            ones_col = wp.tile([P, 1], BF16, name="ones", tag="ones")
            ones_row = wp.tile([1, P], BF16, name="onesr", tag="onesr")
            # per-head Z in bf16 [32, h, 1024]
            zrb = wp.tile([HD, NH, T], BF16, name="zrb", tag="zrb")
            s1row = wp.tile([1, NH * T], BF16, name="s1row", tag="s1row")
            ssr = wp.tile([1, NH], F32, name="ssr", tag="ssr")
            vpe = wp.tile([1, NH], F32, name="vpe", tag="vpe")
            itmp = wp.tile([1, NH], I32, name="itmp", tag="itmp")
            ftmp = wp.tile([1, NH], F32, name="ftmp", tag="ftmp")
            alpha = wp.tile([1, NH], F32, name="alpha", tag="alpha")
            biash = wp.tile([1, NH], BF16, name="biash", tag="biash")
            vs_row = wp.tile([1, C], F32R, name="vsrow", tag="vsrow")
            vs_dh = wp.tile([HD, NH], F32R, name="vsdh", tag="vsdh")
            wtld = wp.tile([P, 2, NH], F32, name="wtld", tag="wtld")
            u_sb = wp.tile([P, 2, NH], F32, name="u", tag="u")
            y_sb = [wp.tile([P, T], F32, name=f"y{i}", tag=f"y{i}") for i in range(2)]


            def _one_rep():
                # ---- phase 0: loads (q/k-critical first) ----
                for i in range(2):
                    nc.sync.dma_start(wqt_sb[i][:], wqt_d[i * P : (i + 1) * P, :])
                    nc.sync.dma_start(xf_sb[i][:], xf_d[i * P : (i + 1) * P, :])
                    nc.sync.dma_start(wkt_sb[i][:], wkt_d[i * P : (i + 1) * P, :])
                for i in range(2):
                    nc.sync.dma_start(wvt_sb[i][:], wvt_d[i * P : (i + 1) * P, :])
                    nc.sync.dma_start(wst_sb[i][:], wst_d[i * P : (i + 1) * P, :])
                    nc.sync.dma_start(bp_sb[i][:], bp_d[i * P : (i + 1) * P, :])
                nc.sync.dma_start(wpt_sb[:], wpt_d[:])
                nc.sync.dma_start(wpct_sb[:], wpct_d[:])
                nc.sync.dma_start(gamma_sb[:], gamma_d[:])
                nc.sync.dma_start(beta_sb[:], beta_d[:])
                nc.vector.memset(ones_col[:], 1.0)
                nc.vector.memset(ones_row[:], 1.0)
                # ones columns inside vt (lhsT column 32 of each head block)
                for tm8 in range(8):
                    vt3 = vt_sb[:, tm8, :].rearrange("p (h e) -> p h e", e=33)
                    nc.vector.tensor_copy(
                        vt3[:, :, 32:33], ones_col[:, 0:1].broadcast_to((P, 8, 1))
                    )

                # ---- phase 1: projections ----
                with tc.tile_pool(name="psA", bufs=2, space=bass.MemorySpace.PSUM) as psA:
                    for wt, dst in ((wqt_sb, q_sb), (wkt_sb, k_sb)):
                        for co in range(2):
                            for tn in range(2):
                                pq = psA.tile([P, 512], F32, name="qk", tag="qk")
                                for kc in range(2):
                                    nc.tensor.matmul(
                                        pq[:],
                                        wt[kc][:, co * P : (co + 1) * P],
                                        xf_sb[kc][:, tn * 512 : (tn + 1) * 512],
                                        start=(kc == 0),
                                        stop=(kc == 1),
                                    )
                                nc.scalar.activation(
                                    dst[co][:, tn * 512 : (tn + 1) * 512], pq[:], AF.Copy
                                )
                    # v^T = xf^T @ Wv^T, written per t-chunk with head-stride 33
                    pvs = psA.tile([1, C], F32, name="vs", tag="vs", bufs=1)
                    for tm in range(8):
                        pv = psA.tile([P, C], F32, name="vt", tag="vt")
                        for kc in range(2):
                            nc.tensor.matmul(
                                pv[:],
                                xf_sb[kc][:, tm * P : (tm + 1) * P],
                                wvt_sb[kc][:],
                                start=(kc == 0),
                                stop=(kc == 1),
                            )
                        src = pv[:].rearrange("p (h d) -> p h d", h=NH)
                        dst3 = vt_sb[:, tm, :].rearrange("p (h e) -> p h e", e=33)
                        nc.scalar.activation(dst3[:, :, 0:32], src[:], AF.Copy)
                        nc.tensor.matmul(
                            pvs[:],
                            ones_col[:],
                            dst3[:, :, 0:32].rearrange("p h d -> p d h"),
                            start=(tm == 0),
                            stop=(tm == 7),
                            skip_group_check=True,
                        )
                    nc.scalar.activation(vs_row[:], pvs[:], AF.Copy)
                    # vs_dh [32, 8] <- vs_row [1, 256] partition-unflatten
                    nc.sync.dma_start(vs_dh[:], vs_row[:])
                    # wtld[:, oc, :] = wpct[:, oc]^T @ vs_dh   [128, 2, 8]
                    pw = psA.tile([P, 2, NH], F32, name="pw", tag="pw", bufs=1)
                    for oc in range(2):
                        nc.tensor.matmul(
                            pw[:, oc, :],
                            wpct_sb[:, oc * P : (oc + 1) * P],
                            vs_dh[:],
                            start=True,
                            stop=True,
                        )
                    nc.scalar.activation(wtld[:], pw[:], AF.Copy)

                # ---- phase 2: streaming attention + interleaved stats/tails
                qts = [None] * NH
                pavs = [None] * NH
                rinvs = [None] * NH

                def mk_qt(h):
                    # ACT Copy with per-partition scale: qt = q * wst[:, h]
                    qt = sp.tile([P, 2, T], F32R, name="qt", tag="qt")
                    for kc in range(2):
                        if "qtdve" in KVAR:
                            nc.vector.tensor_scalar_mul(
                                qt[:, kc, :], q_sb[kc][:], wst_sb[kc][:, h : h + 1]
                            )
                        else:
                            nc.scalar.activation(
                                qt[:, kc, :], q_sb[kc][:], AF.Copy,
                                scale=wst_sb[kc][:, h : h + 1],
                            )
                    qts[h] = qt

                def stats_s0(h):
                    rinv = sp.tile([1, T], F32, name="rinv", tag="rinv")
                    nc.vector.reciprocal(rinv[:], pavs[h][32:33, :])
                    rinvs[h] = rinv

                def stats_s1(h):
                    tt1 = sp.tile([1, T], F32, name="tt1", tag="tt1")
                    nc.vector.tensor_mul(tt1[:], pavs[h][64:65, :], rinvs[h][:])
                    tt2 = sp.tile([1, T], F32, name="tt2", tag="tt2")
                    nc.vector.scalar_tensor_tensor(
                        tt2[:], tt1[:], 1.0, rinvs[h][:],
                        op0=ALU.mult, op1=ALU.mult,
                        accum_out=ssr[0:1, h : h + 1],
                    )
                    nc.vector.tensor_scalar(
                        vpe[0:1, h : h + 1], ssr[0:1, h : h + 1],
                        1.0 / (T * T), EPS - 1.0 / (T * T),
                        op0=ALU.mult, op1=ALU.add,
                    )

                def stats_s2(h):
                    # alpha = gamma * rsqrt(vpe): Newton rsqrt on DVE
                    hs = slice(h, h + 1)
                    v_ = vpe[0:1, hs]
                    i_ = itmp[0:1, hs]
                    f_ = ftmp[0:1, hs]
                    y_ = i_.bitcast(F32)
                    nc.vector.tensor_scalar(
                        i_, v_.bitcast(I32), 1, None, op0=ALU.logical_shift_right
                    )
                    nc.vector.tensor_scalar(
                        i_, i_, -1, MAGIC, op0=ALU.mult, op1=ALU.add
                    )
                    for _ in range(2):
                        nc.vector.tensor_mul(f_, v_, y_)
                        nc.vector.tensor_mul(f_, f_, y_)
                        nc.vector.tensor_scalar(
                            f_, f_, -0.5, 1.5, op0=ALU.mult, op1=ALU.add
                        )
                        nc.vector.tensor_mul(i_.bitcast(F32), y_, f_)
                    nc.vector.tensor_mul(
                        alpha[0:1, hs], y_, gamma_sb[0:1, hs]
                    )
                    nc.vector.scalar_tensor_tensor(
                        biash[0:1, hs], alpha[0:1, hs], -1.0 / T, beta_sb[0:1, hs],
                        op0=ALU.mult, op1=ALU.add,
                    )

                def stats_s3(h):
                    # s1 row (bf16) at partition 0
                    nc.vector.tensor_scalar_mul(
                        s1row[0:1, h * T : (h + 1) * T], rinvs[h][:],
                        alpha[0:1, h : h + 1],
                    )

                def stats_s4(h):
                    # u[:, oc, h] = wtld * biash_h + bp; biash broadcast to
                    # 128 partitions via a tiny PE matmul (ones_row^T @ biash)
                    pbb = psS.tile([P, 512], F32, name="s", tag="s")
                    nc.tensor.matmul(
                        pbb[:, 0:1], ones_row[0:1, :], biash[0:1, h : h + 1],
                        start=True, stop=True,
                    )
                    for oc in range(2):
                        nc.vector.tensor_mul(
                            u_sb[:, oc, h : h + 1], wtld[:, oc, h : h + 1],
                            pbb[:, 0:1],
                        )
                        nc.vector.tensor_scalar_add(
                            u_sb[:, oc, h : h + 1], u_sb[:, oc, h : h + 1],
                            bp_sb[oc][:, 0:1],
                        )

                def tail_t0(h):
                    # broadcast s1 row to 32 partitions via ones_row matmul
                    # (psS "s"-tag tiles: no extra PSUM banks), then scale Z
                    zs = sp.tile([HD, T], BF16, name="zs", tag="zs")
                    for qh in range(2):
                        pb = psS.tile([P, 512], F32, name="s", tag="s")
                        nc.tensor.matmul(
                            pb[0:32, :],
                            ones_row[0:1, 0:32],
                            s1row[0:1, h * T + qh * 512 : h * T + (qh + 1) * 512],
                            start=True, stop=True,
                        )
                        nc.vector.tensor_mul(
                            zs[:, qh * 512 : (qh + 1) * 512],
                            zrb[:, h, qh * 512 : (qh + 1) * 512],
                            pb[0:32, :],
                        )
                    return zs

                def tail_t1(h, zs):
                    py = psB.tile([P, 2, P], F32, name="py", tag="py", bufs=1)
                    zs8 = zs[:].rearrange("p (m j) -> p m j", j=8)
                    for oc in range(2):
                        for j in range(8):
                            nc.tensor.matmul(
                                py[:, oc, :],
                                wpt_sb[:, j, oc * P : (oc + 1) * P],
                                zs8[:, :, j],
                                start=(j == 0),
                                stop=(j == 7),
                            )
                        nc.vector.tensor_scalar_add(
                            y_sb[oc][:, h * P : (h + 1) * P],
                            py[:, oc, :],
                            u_sb[:, oc, h : h + 1],
                        )
                        nc.sync.dma_start(
                            yt_d[oc * P : (oc + 1) * P, h * P : (h + 1) * P],
                            y_sb[oc][:, h * P : (h + 1) * P],
                        )

                with (
                    tc.tile_pool(name="psS", bufs=3, space=bass.MemorySpace.PSUM) as psS,
                    tc.tile_pool(name="psB", bufs=2, space=bass.MemorySpace.PSUM) as psB,
                ):
                    mk_qt(0)
                    zss = [None] * NH
                    for h in range(NH):
                        pav = psB.tile([65, T], F32, name="av", tag="av")
                        pavs[h] = pav
                        for tm in range(8):
                            et = sp.tile([P, T], BF16, name="E", tag="E", bufs=3)
                            for qh in range(2):
                                ps = psS.tile([P, 512], F32, name="s", tag="s")
                                for kc in range(2):
                                    nc.tensor.matmul(
                                        ps[:],
                                        k_sb[kc][:, tm * P : (tm + 1) * P],
                                        qts[h][:, kc, qh * 512 : (qh + 1) * 512],
                                        start=(kc == 0),
                                        stop=(kc == 1),
                                    )
                                nc.scalar.activation(
                                    et[:, qh * 512 : (qh + 1) * 512], ps[:],
                                    AF.Exp, scale=SCALE,
                                )
                            # interleaved work for previous heads
                            st = "notail" not in KVAR
                            if tm == 0 and h >= 1 and st:
                                stats_s0(h - 1)
                            if tm == 1 and h >= 2 and st:
                                zss[h - 2] = tail_t0(h - 2)
                            if tm == 2 and h + 1 < NH:
                                mk_qt(h + 1)
                            if tm == 2 and h >= 1 and st:
                                stats_s1(h - 1)
                            if tm == 3 and h >= 2 and st:
                                tail_t1(h - 2, zss[h - 2])
                            if tm == 4 and h >= 1 and st:
                                stats_s2(h - 1)
                            if tm == 5 and h >= 1 and st:
                                stats_s3(h - 1)
                            if tm == 6 and h >= 1 and st:
                                stats_s4(h - 1)
                            sq = None
                            if "nosq" not in KVAR:
                                sq = sp.tile([P, T], BF16, name="SQ", tag="SQ", bufs=3)
                                nc.vector.tensor_mul(sq[:], et[:], et[:])
                            for qh in range(2):
                                nc.tensor.matmul(
                                    pav[0:33, qh * 512 : (qh + 1) * 512],
                                    vt_sb[:, tm, 33 * h : 33 * h + 33],
                                    et[:, qh * 512 : (qh + 1) * 512],
                                    start=(tm == 0),
                                    stop=(tm == 7),
                                    skip_group_check=True,
                                )
                            if sq is not None:
                                for qh in range(2):
                                    nc.tensor.matmul(
                                        pav[64:65, qh * 512 : (qh + 1) * 512],
                                        ones_col[:],
                                        sq[:, qh * 512 : (qh + 1) * 512],
                                        start=(tm == 0),
                                        stop=(tm == 7),
                                        skip_group_check=True,
                                    )
                        # Z -> SBUF bf16 (DVE; keeps ACT exp-only)
                        if "nozrb" not in KVAR:
                            nc.vector.tensor_copy(zrb[:, h, :], pav[0:32, :])
                    if "notail" not in KVAR:
                        # drain: stats(7), tails 6 and 7
                        stats_s0(7)
                        zss[6] = tail_t0(6)
                        stats_s1(7)
                        tail_t1(6, zss[6])
                        stats_s2(7)
                        stats_s3(7)
                        stats_s4(7)
                        zss[7] = tail_t0(7)
                        tail_t1(7, zss[7])
                    else:
                        for oc in range(2):
                            nc.sync.dma_start(
                                yt_d[oc * P : (oc + 1) * P, :],
                                q_sb[oc][:].bitcast(F32),
                            )

            for _rep in range(reps):
                _one_rep()

    _split_excess_waits(nc)
    return nc


def _host_inputs(x, Wq, Wk, Wv, w_head, gamma, beta, Wp, bp):
    f = np.float32
    wpt = np.ascontiguousarray(np.asarray(Wp, f).T.reshape(8, HD, C).transpose(1, 0, 2))
    common = {
        "wqt": np.ascontiguousarray(np.asarray(Wq, f).T),
        "wkt": np.ascontiguousarray(np.asarray(Wk, f).T),
        "wvt": np.ascontiguousarray(np.asarray(Wv, f).T),
        "wst": np.ascontiguousarray(np.repeat(np.asarray(w_head, f), HD, axis=1).T),
        "wpt": wpt.astype(ml_dtypes.bfloat16),
        "wpct": np.ascontiguousarray(wpt.sum(1)),
        "bp": np.ascontiguousarray(np.asarray(bp, f).reshape(C, 1)),
        "gamma": np.ascontiguousarray(np.asarray(gamma, f).reshape(1, NH)),
        "beta": np.ascontiguousarray(np.asarray(beta, f).reshape(1, NH)),
    }
    xs = np.asarray(x, f).reshape(B, C, T)
    return [
        {"xf": np.ascontiguousarray(xs[b]), **common} for b in range(B)
    ]


_NC_CACHE = {}


def _get_nc(reps=1):
    if reps not in _NC_CACHE:
        _NC_CACHE[reps] = build_bass(reps=reps)
    return _NC_CACHE[reps]


def run(inputs, trace=False):
    nc = _get_nc()
    in_maps = _host_inputs(**inputs)
    res = run_bass_kernel_spmd(
        nc, in_maps, core_ids=list(range(N_CORES)), trace=trace
    )
    y = np.stack([res.results[b]["yt"] for b in range(B)], axis=0)
    return y.reshape(B, C, 32, 32).astype(np.float32), res


def _build_sharded(reps=1):
    """Replicate bass2jax.run_bass_via_pjrt but return a reusable callable
    (no donation) so device execution can be timed over many iterations."""
    import jax
    from jax.sharding import Mesh, PartitionSpec
    from jax.experimental.shard_map import shard_map
    from concourse import bass2jax

    nc = _get_nc(reps)
    bass2jax.install_neuronx_cc_hook()
    part_name = nc.partition_id_tensor.name if nc.partition_id_tensor else None
    in_names, out_names, out_avals = [], [], []
    for alloc in nc.m.functions[0].allocations:
        if not isinstance(alloc, mybir.MemoryLocationSet):
            continue
        name = alloc.memorylocations[0].name
        if alloc.kind == "ExternalInput":
            if name == part_name:
                continue
            in_names.append(name)
        elif alloc.kind == "ExternalOutput":
            out_names.append(name)
            out_avals.append(
                jax.core.ShapedArray(
                    tuple(alloc.tensor_shape), mybir.dt.np(alloc.dtype)
                )
            )
    n_params = len(in_names)
    all_in = in_names + out_names
    if part_name is not None:
        all_in = all_in + [part_name]

    def _body(*args):
        operands = list(args)
        if part_name is not None:
            operands.append(bass2jax.partition_id_tensor())
        outs = bass2jax._bass_exec_p.bind(
            *operands,
            out_avals=tuple(out_avals),
            in_names=tuple(all_in),
            out_names=tuple(out_names),
            lowering_input_output_aliases=(),
            sim_require_finite=True,
            sim_require_nnan=True,
            nc=nc,
        )
        return tuple(outs)

    devices = jax.devices()[:N_CORES]
    mesh = Mesh(np.asarray(devices), ("core",))
    nouts = len(out_names)
    sharded = jax.jit(
        shard_map(
            _body,
            mesh=mesh,
            in_specs=(PartitionSpec("core"),) * (n_params + nouts),
            out_specs=(PartitionSpec("core"),) * nouts,
            check_rep=False,
        ),
        keep_unused=True,
    )
    return sharded, mesh, in_names, out_names, out_avals


def timed_run(inputs, iters=20, reps=1):
    import time
    import jax
    from jax.sharding import NamedSharding, PartitionSpec

    sharded, mesh, in_names, out_names, out_avals = _build_sharded(reps)
    in_maps = _host_inputs(**inputs)
    sh = NamedSharding(mesh, PartitionSpec("core"))
    dev_in = [
        jax.device_put(
            np.concatenate([in_maps[c][n] for c in range(N_CORES)], axis=0), sh
        )
        for n in in_names
    ]
    dev_zero = [
        jax.device_put(
            np.zeros((N_CORES * a.shape[0], *a.shape[1:]), a.dtype), sh
        )
        for a in out_avals
    ]
    out = sharded(*dev_in, *dev_zero)
    jax.block_until_ready(out)
    # blocking per-call (includes full dispatch round trip)
    times = []
    for _ in range(max(3, iters // 4)):
        t0 = time.perf_counter()
        out = sharded(*dev_in, *dev_zero)
        jax.block_until_ready(out)
        times.append(time.perf_counter() - t0)
    # pipelined: submit all, block once -> amortizes host/axon dispatch
    t0 = time.perf_counter()
    outs = [sharded(*dev_in, *dev_zero) for _ in range(iters)]
    jax.block_until_ready(outs)
    pipelined = (time.perf_counter() - t0) / iters
    times.append(pipelined)
    print(f"pipelined per-call: {pipelined * 1e9:.0f} ns")
    y = np.asarray(outs[-1][out_names.index("yt")]).reshape(N_CORES, C, T)
    return y.reshape(B, C, 32, 32).astype(np.float32), times


def kernel(**inputs):
    y, _ = run(inputs, trace=False)
    return y


def numpy_check():
    """CoreSim single-core check against a numpy reference (core 0 data)."""
    from concourse.bass_interp import CoreSim

    rng = np.random.default_rng(0)
    x = rng.standard_normal((B, C, 32, 32), np.float32)
    Wq = (rng.standard_normal((C, C)) * 0.05).astype(np.float32)
    Wk = (rng.standard_normal((C, C)) * 0.05).astype(np.float32)
    Wv = (rng.standard_normal((C, C)) * 0.05).astype(np.float32)
    w_head = (rng.standard_normal((NH, NH)) * 0.3).astype(np.float32)
    gamma = rng.uniform(0.5, 1.5, NH).astype(np.float32)
    beta = (rng.standard_normal(NH) * 0.1).astype(np.float32)
    Wp = (rng.standard_normal((C, C)) * 0.05).astype(np.float32)
    bp = (rng.standard_normal(C) * 0.05).astype(np.float32)
    inputs = dict(
        x=x, Wq=Wq, Wk=Wk, Wv=Wv, w_head=w_head, gamma=gamma, beta=beta,
        Wp=Wp, bp=bp,
    )

    def ref_np(x, Wq, Wk, Wv, w_head, gamma, beta, Wp, bp):
        Bn, Cn, H, W = x.shape
        Tn = H * W
        hd = Cn // NH
        sc = float(hd) ** -0.5
        xf = x.reshape(Bn, Cn, Tn).astype(np.float64)
        q = np.einsum("oc,bct->bot", Wq, xf).reshape(Bn, NH, hd, Tn)
        k = np.einsum("oc,bct->bot", Wk, xf).reshape(Bn, NH, hd, Tn)
        v = np.einsum("oc,bct->bot", Wv, xf).reshape(Bn, NH, hd, Tn)
        s = np.einsum("bhdq,bhdt->bhqt", q, k) * sc
        s = np.einsum("hg,bgqt->bhqt", w_head.astype(np.float64), s)
        s = s - s.max(axis=-1, keepdims=True)
        e = np.exp(s)
        a = e / e.sum(-1, keepdims=True)
        mean = a.mean(axis=(2, 3), keepdims=True)
        var = a.var(axis=(2, 3), keepdims=True)
        g = gamma.astype(np.float64)[None, :, None, None]
        bt = beta.astype(np.float64)[None, :, None, None]
        a = (a - mean) / np.sqrt(var + EPS) * g + bt
        out = np.einsum("bhqt,bhdt->bhqd", a, v)
        y = out.reshape(Bn, Tn, Cn)
        y = np.einsum("btc,oc->bto", y, Wp.astype(np.float64)) + bp
        return y.transpose(0, 2, 1).reshape(Bn, Cn, H, W)

    expected = ref_np(**inputs)[0]  # core 0

    nc = _get_nc()
    in_maps = _host_inputs(**inputs)
    sim = CoreSim(nc, trace=False)
    for name, arr in in_maps[0].items():
        sim.tensor(name)[:] = arr
    sim.simulate(check_with_hw=False)
    got = np.array(sim.tensor("yt")).reshape(C, 32, 32)
    err = np.abs(got - expected) / (np.abs(expected) + 1e-3)
    print("max rel err (sim vs numpy f64):", err.max())
    print("mean rel err:", err.mean())
    return err.max()


if __name__ == "__main__":
    numpy_check()



# revision 6
# speedup vs baseline: 1.6424x; 1.2793x over previous
"""CMHSA Trainium2 kernel v2: per-head fused tail, direct-PSUM stats.

Per core (B=8 -> 8 cores data-parallel):
  q = Wq@xf, k = Wk@xf, v = Wv@xf            (C x T)
  s^T[h] = k^T @ (q . wst_h)  (K=256, w_head folded into q scaling via ACT
           Copy with per-partition scale)
  E = exp(sc * s^T) bf16 [t, q];  sq = E*E bf16 (DVE 2x)
  pav rows: 0:32 Z = v^T E (lhsT = [v^T | 1] -> row 32 = r), 64 rs2 = 1^T sq
  per-head stats read pav rows straight from PSUM (32-aligned partition
  shifts are legal): rinv = 1/r; ss = sum_q rs2*rinv^2; var = (ss-1)/T^2;
  alpha = gamma * rsqrt(var+eps)  (Newton rsqrt on DVE, no ACT table switch)
  s1 = alpha*rinv -> s1row (bf16, partition 0); biash = beta - alpha/T
  tail: gpsimd bcast s1 -> [32,T]; zs = zrb*s1 (bf16 2x);
    y[:, 128h:128h+128] = sum_j wpt_j^T @ zs[:, j::8] + u[:, h]; DMA out
  u[:, oc, h] = wtld[:, oc, h]*biash_h + bp   (wtld = wpct^T vs_dh, phase 1)
  Stats for head h are spread over head h+1's tm slots; tail of h runs
  during head h+2. The DVE/ACT/Pool queues stay off the PE critical path.
"""

import math
import os
import warnings

warnings.filterwarnings("ignore")

import numpy as np
import ml_dtypes

import concourse.bass as bass
import concourse.mybir as mybir
import concourse.tile as tile
from concourse import library_config
from concourse.bass_utils import run_bass_kernel_spmd

B, C, T, NH, HD, P = 8, 256, 1024, 8, 32, 128
EPS = 1e-5
SCALE = 1.0 / math.sqrt(HD)
F32 = mybir.dt.float32
F32R = mybir.dt.float32r
BF16 = mybir.dt.bfloat16
I32 = mybir.dt.int32
AF = mybir.ActivationFunctionType
ALU = mybir.AluOpType
N_CORES = 8
MAGIC = 0x5F3759DF
KVAR = set(os.environ.get("KVAR", "").split(","))


def _split_excess_waits(nc, max_waits=1):
    """This walrus build rejects >2 sem-waits on one instruction
    ("Too many sync wait commands" in setupSyncWait). Hoist excess waits
    onto same-engine NoOps inserted right before the offending instruction."""
    for f in nc.m.functions:
        for blk in f.blocks:
            insts = list(blk.instructions)
            out, changed = [], False
            for inst in insts:
                si = inst.sync_info
                waits = list(si.on_wait) if si and si.on_wait else []
                if len(waits) > max_waits:
                    extra, keep = waits[:-max_waits], waits[-max_waits:]
                    for w in extra:
                        nop = mybir.InstNoOp(
                            name=f"I-sw-{nc.next_id()}",
                            ins=[],
                            outs=[],
                            sync_info=mybir.SyncInfo(on_wait=[w], on_update=[]),
                            engine=inst.engine,
                        )
                        nc.register_instruction(nop)
                        out.append(nop)
                    inst.sync_info = mybir.SyncInfo(
                        on_wait=keep, on_update=list(si.on_update or [])
                    )
                    changed = True
                out.append(inst)
            if changed:
                blk.instructions = out


def build_bass(reps=1):
    nc = bass.Bass("TRN2", target_bir_lowering=False, debug=False)

    xf_d = nc.dram_tensor("xf", [C, T], F32R, kind="ExternalInput")
    wqt_d = nc.dram_tensor("wqt", [C, C], F32R, kind="ExternalInput")
    wkt_d = nc.dram_tensor("wkt", [C, C], F32R, kind="ExternalInput")
    wvt_d = nc.dram_tensor("wvt", [C, C], F32R, kind="ExternalInput")
    wst_d = nc.dram_tensor("wst", [C, NH], F32, kind="ExternalInput")
    wpt_d = nc.dram_tensor("wpt", [HD, 8, C], BF16, kind="ExternalInput")
    wpct_d = nc.dram_tensor("wpct", [HD, C], F32R, kind="ExternalInput")
    bp_d = nc.dram_tensor("bp", [C, 1], F32, kind="ExternalInput")
    gamma_d = nc.dram_tensor("gamma", [1, NH], F32, kind="ExternalInput")
    beta_d = nc.dram_tensor("beta", [1, NH], F32, kind="ExternalInput")
    yt_d = nc.dram_tensor("yt", [C, T], F32, kind="ExternalOutput")

    with tile.TileContext(nc) as tc:
        with (
            tc.tile_pool(name="w", bufs=1) as wp,
            tc.tile_pool(name="stream", bufs=2) as sp,
        ):
            # ---- persistent SBUF tensors ----
            xf_sb = [wp.tile([P, T], F32R, name=f"xf{i}", tag=f"xf{i}") for i in range(2)]
            wqt_sb = [wp.tile([P, C], F32R, name=f"wqt{i}", tag=f"wqt{i}") for i in range(2)]
            wkt_sb = [wp.tile([P, C], F32R, name=f"wkt{i}", tag=f"wkt{i}") for i in range(2)]
            wvt_sb = [wp.tile([P, C], F32R, name=f"wvt{i}", tag=f"wvt{i}") for i in range(2)]
            wst_sb = [wp.tile([P, NH], F32, name=f"wst{i}", tag=f"wst{i}") for i in range(2)]
            wpt_sb = wp.tile([HD, 8, C], BF16, name="wptj", tag="wptj")
            wpct_sb = wp.tile([HD, C], F32R, name="wpct", tag="wpct")
            bp_sb = [wp.tile([P, 1], F32, name=f"bp{i}", tag=f"bp{i}") for i in range(2)]
            gamma_sb = wp.tile([1, NH], F32, name="gamma", tag="gamma")
            beta_sb = wp.tile([1, NH], F32, name="beta", tag="beta")
            q_sb = [wp.tile([P, T], F32R, name=f"q{i}", tag=f"q{i}") for i in range(2)]
            k_sb = [wp.tile([P, T], F32R, name=f"k{i}", tag=f"k{i}") for i in range(2)]
            # v^T tiles: per t-chunk tm, 8 heads x (32 cols + ones col), bf16
            ET_DT = F32R if "etf32" in KVAR else BF16
            vt_sb = wp.tile([P, 8, NH * 33], ET_DT, name="vt", tag="vt")
            ones_col = wp.tile([P, 1], ET_DT, name="ones", tag="ones")
            ones_row = wp.tile([1, P], BF16, name="onesr", tag="onesr")
            # per-head Z in bf16 [32, h, 1024]
            zrb = wp.tile([HD, NH, T], BF16, name="zrb", tag="zrb")
            s1row = wp.tile([1, NH * T], BF16, name="s1row", tag="s1row")
            ssr = wp.tile([1, NH], F32, name="ssr", tag="ssr")
            vpe = wp.tile([1, NH], F32, name="vpe", tag="vpe")
            itmp = wp.tile([1, NH], I32, name="itmp", tag="itmp")
            ftmp = wp.tile([1, NH], F32, name="ftmp", tag="ftmp")
            alpha = wp.tile([1, NH], F32, name="alpha", tag="alpha")
            biash = wp.tile([1, NH], BF16, name="biash", tag="biash")
            vs_row = wp.tile([1, C], F32R, name="vsrow", tag="vsrow")
            vs_dh = wp.tile([HD, NH], F32R, name="vsdh", tag="vsdh")
            wtld = wp.tile([P, 2, NH], F32, name="wtld", tag="wtld")
            u_sb = wp.tile([P, 2, NH], F32, name="u", tag="u")
            y_sb = [wp.tile([P, T], F32, name=f"y{i}", tag=f"y{i}") for i in range(2)]


            def _one_rep():
                # ---- phase 0: loads (q/k-critical first) ----
                for i in range(2):
                    nc.sync.dma_start(wqt_sb[i][:], wqt_d[i * P : (i + 1) * P, :])
                    nc.sync.dma_start(xf_sb[i][:], xf_d[i * P : (i + 1) * P, :])
                    nc.sync.dma_start(wkt_sb[i][:], wkt_d[i * P : (i + 1) * P, :])
                for i in range(2):
                    nc.sync.dma_start(wvt_sb[i][:], wvt_d[i * P : (i + 1) * P, :])
                    nc.sync.dma_start(wst_sb[i][:], wst_d[i * P : (i + 1) * P, :])
                    nc.sync.dma_start(bp_sb[i][:], bp_d[i * P : (i + 1) * P, :])
                nc.sync.dma_start(wpt_sb[:], wpt_d[:])
                nc.sync.dma_start(wpct_sb[:], wpct_d[:])
                nc.sync.dma_start(gamma_sb[:], gamma_d[:])
                nc.sync.dma_start(beta_sb[:], beta_d[:])
                nc.vector.memset(ones_col[:], 1.0)
                nc.vector.memset(ones_row[:], 1.0)
                # ones columns inside vt (lhsT column 32 of each head block)
                for tm8 in range(8):
                    vt3 = vt_sb[:, tm8, :].rearrange("p (h e) -> p h e", e=33)
                    nc.vector.tensor_copy(
                        vt3[:, :, 32:33], ones_col[:, 0:1].broadcast_to((P, 8, 1))
                    )

                # ---- phase 1: projections ----
                with tc.tile_pool(name="psA", bufs=2, space=bass.MemorySpace.PSUM) as psA:
                    for wt, dst in ((wqt_sb, q_sb), (wkt_sb, k_sb)):
                        for co in range(2):
                            for tn in range(2):
                                pq = psA.tile([P, 512], F32, name="qk", tag="qk")
                                for kc in range(2):
                                    nc.tensor.matmul(
                                        pq[:],
                                        wt[kc][:, co * P : (co + 1) * P],
                                        xf_sb[kc][:, tn * 512 : (tn + 1) * 512],
                                        start=(kc == 0),
                                        stop=(kc == 1),
                                    )
                                nc.scalar.activation(
                                    dst[co][:, tn * 512 : (tn + 1) * 512], pq[:], AF.Copy
                                )
                    # v^T = xf^T @ Wv^T, written per t-chunk with head-stride 33
                    pvs = psA.tile([1, C], F32, name="vs", tag="vs", bufs=1)
                    for tm in range(8):
                        pv = psA.tile([P, C], F32, name="vt", tag="vt")
                        for kc in range(2):
                            nc.tensor.matmul(
                                pv[:],
                                xf_sb[kc][:, tm * P : (tm + 1) * P],
                                wvt_sb[kc][:],
                                start=(kc == 0),
                                stop=(kc == 1),
                            )
                        src = pv[:].rearrange("p (h d) -> p h d", h=NH)
                        dst3 = vt_sb[:, tm, :].rearrange("p (h e) -> p h e", e=33)
                        nc.scalar.activation(dst3[:, :, 0:32], src[:], AF.Copy)
                        nc.tensor.matmul(
                            pvs[:],
                            ones_col[:],
                            dst3[:, :, 0:32].rearrange("p h d -> p d h"),
                            start=(tm == 0),
                            stop=(tm == 7),
                            skip_group_check=True,
                        )
                    nc.scalar.activation(vs_row[:], pvs[:], AF.Copy)
                    # vs_dh [32, 8] <- vs_row [1, 256] partition-unflatten
                    nc.sync.dma_start(vs_dh[:], vs_row[:])
                    # wtld[:, oc, :] = wpct[:, oc]^T @ vs_dh   [128, 2, 8]
                    pw = psA.tile([P, 2, NH], F32, name="pw", tag="pw", bufs=1)
                    for oc in range(2):
                        nc.tensor.matmul(
                            pw[:, oc, :],
                            wpct_sb[:, oc * P : (oc + 1) * P],
                            vs_dh[:],
                            start=True,
                            stop=True,
                        )
                    nc.scalar.activation(wtld[:], pw[:], AF.Copy)

                # ---- phase 2: streaming attention + interleaved stats/tails
                qts = [None] * NH
                pavs = [None] * NH
                rinvs = [None] * NH

                def mk_qt(h):
                    # ACT Copy with per-partition scale: qt = q * wst[:, h]
                    qt = sp.tile([P, 2, T], F32R, name="qt", tag="qt")
                    for kc in range(2):
                        if "qtdve" in KVAR:
                            nc.vector.tensor_scalar_mul(
                                qt[:, kc, :], q_sb[kc][:], wst_sb[kc][:, h : h + 1]
                            )
                        else:
                            nc.scalar.activation(
                                qt[:, kc, :], q_sb[kc][:], AF.Copy,
                                scale=wst_sb[kc][:, h : h + 1],
                            )
                    qts[h] = qt

                def stats_s0(h):
                    rinv = sp.tile([1, T], F32, name="rinv", tag="rinv")
                    nc.vector.reciprocal(rinv[:], pavs[h][32:33, :])
                    rinvs[h] = rinv

                def stats_s1(h):
                    tt1 = sp.tile([1, T], F32, name="tt1", tag="tt1")
                    nc.vector.tensor_mul(tt1[:], pavs[h][64:65, :], rinvs[h][:])
                    tt2 = sp.tile([1, T], F32, name="tt2", tag="tt2")
                    nc.vector.scalar_tensor_tensor(
                        tt2[:], tt1[:], 1.0, rinvs[h][:],
                        op0=ALU.mult, op1=ALU.mult,
                        accum_out=ssr[0:1, h : h + 1],
                    )
                    nc.vector.tensor_scalar(
                        vpe[0:1, h : h + 1], ssr[0:1, h : h + 1],
                        1.0 / (T * T), EPS - 1.0 / (T * T),
                        op0=ALU.mult, op1=ALU.add,
                    )

                def stats_s2(h):
                    # alpha = gamma * rsqrt(vpe): Newton rsqrt on DVE
                    hs = slice(h, h + 1)
                    v_ = vpe[0:1, hs]
                    i_ = itmp[0:1, hs]
                    f_ = ftmp[0:1, hs]
                    y_ = i_.bitcast(F32)
                    nc.vector.tensor_scalar(
                        i_, v_.bitcast(I32), 1, None, op0=ALU.logical_shift_right
                    )
                    nc.vector.tensor_scalar(
                        i_, i_, -1, MAGIC, op0=ALU.mult, op1=ALU.add
                    )
                    for _ in range(2):
                        nc.vector.tensor_mul(f_, v_, y_)
                        nc.vector.tensor_mul(f_, f_, y_)
                        nc.vector.tensor_scalar(
                            f_, f_, -0.5, 1.5, op0=ALU.mult, op1=ALU.add
                        )
                        nc.vector.tensor_mul(i_.bitcast(F32), y_, f_)
                    nc.vector.tensor_mul(
                        alpha[0:1, hs], y_, gamma_sb[0:1, hs]
                    )
                    nc.vector.scalar_tensor_tensor(
                        biash[0:1, hs], alpha[0:1, hs], -1.0 / T, beta_sb[0:1, hs],
                        op0=ALU.mult, op1=ALU.add,
                    )

                def stats_s3(h):
                    # s1 row (bf16) at partition 0
                    nc.vector.tensor_scalar_mul(
                        s1row[0:1, h * T : (h + 1) * T], rinvs[h][:],
                        alpha[0:1, h : h + 1],
                    )

                def stats_s4(h):
                    # u[:, oc, h] = wtld * biash_h + bp; biash broadcast to
                    # 128 partitions via a tiny PE matmul (ones_row^T @ biash)
                    pbb = psS.tile([P, 512], F32, name="s", tag="s")
                    nc.tensor.matmul(
                        pbb[:, 0:1], ones_row[0:1, :], biash[0:1, h : h + 1],
                        start=True, stop=True,
                    )
                    for oc in range(2):
                        nc.vector.tensor_mul(
                            u_sb[:, oc, h : h + 1], wtld[:, oc, h : h + 1],
                            pbb[:, 0:1],
                        )
                        nc.vector.tensor_scalar_add(
                            u_sb[:, oc, h : h + 1], u_sb[:, oc, h : h + 1],
                            bp_sb[oc][:, 0:1],
                        )

                def tail_t0(h):
                    # broadcast s1 row to 32 partitions via ones_row matmul
                    # (psS "s"-tag tiles: no extra PSUM banks), then scale Z
                    zs = sp.tile([HD, T], BF16, name="zs", tag="zs")
                    for qh in range(2):
                        pb = psS.tile([P, 512], F32, name="s", tag="s")
                        nc.tensor.matmul(
                            pb[0:32, :],
                            ones_row[0:1, 0:32],
                            s1row[0:1, h * T + qh * 512 : h * T + (qh + 1) * 512],
                            start=True, stop=True,
                        )
                        nc.vector.tensor_mul(
                            zs[:, qh * 512 : (qh + 1) * 512],
                            zrb[:, h, qh * 512 : (qh + 1) * 512],
                            pb[0:32, :],
                        )
                    return zs

                def tail_t1(h, zs):
                    py = psB.tile([P, 2, P], F32, name="py", tag="py", bufs=1)
                    zs8 = zs[:].rearrange("p (m j) -> p m j", j=8)
                    for oc in range(2):
                        for j in range(8):
                            nc.tensor.matmul(
                                py[:, oc, :],
                                wpt_sb[:, j, oc * P : (oc + 1) * P],
                                zs8[:, :, j],
                                start=(j == 0),
                                stop=(j == 7),
                            )
                        nc.vector.tensor_scalar_add(
                            y_sb[oc][:, h * P : (h + 1) * P],
                            py[:, oc, :],
                            u_sb[:, oc, h : h + 1],
                        )
                        nc.sync.dma_start(
                            yt_d[oc * P : (oc + 1) * P, h * P : (h + 1) * P],
                            y_sb[oc][:, h * P : (h + 1) * P],
                        )

                psS_bufs = 2 if "ps1024" in KVAR else 3
                with (
                    tc.tile_pool(name="psS", bufs=psS_bufs, space=bass.MemorySpace.PSUM) as psS,
                    tc.tile_pool(name="psB", bufs=2, space=bass.MemorySpace.PSUM) as psB,
                ):
                    mk_qt(0)
                    zss = [None] * NH
                    for h in range(NH):
                        pav = psB.tile([65, T], F32, name="av", tag="av")
                        pavs[h] = pav
                        for tm in range(8):
                            et = sp.tile([P, T], ET_DT, name="E", tag="E", bufs=3)
                            if "ps1024" in KVAR:
                                ps = psS.tile([P, T], F32, name="s", tag="s")
                                for qh in range(2):
                                    for kc in range(2):
                                        nc.tensor.matmul(
                                            ps[:, qh * 512 : (qh + 1) * 512],
                                            k_sb[kc][:, tm * P : (tm + 1) * P],
                                            qts[h][:, kc, qh * 512 : (qh + 1) * 512],
                                            start=(kc == 0),
                                            stop=(kc == 1),
                                        )
                                nc.scalar.activation(et[:], ps[:], AF.Exp, scale=SCALE)
                            else:
                                for qh in range(2):
                                    ps = psS.tile([P, 512], F32, name="s", tag="s")
                                    for kc in range(2):
                                        nc.tensor.matmul(
                                            ps[:],
                                            k_sb[kc][:, tm * P : (tm + 1) * P],
                                            qts[h][:, kc, qh * 512 : (qh + 1) * 512],
                                            start=(kc == 0),
                                            stop=(kc == 1),
                                        )
                                    nc.scalar.activation(
                                        et[:, qh * 512 : (qh + 1) * 512], ps[:],
                                        AF.Exp, scale=SCALE,
                                    )
                            # interleaved work for previous heads
                            st = "notail" not in KVAR
                            if tm == 0 and h >= 1 and st:
                                stats_s0(h - 1)
                            if tm == 1 and h >= 2 and st:
                                zss[h - 2] = tail_t0(h - 2)
                            if tm == 2 and h + 1 < NH:
                                mk_qt(h + 1)
                            if tm == 2 and h >= 1 and st:
                                stats_s1(h - 1)
                            if tm == 3 and h >= 2 and st:
                                tail_t1(h - 2, zss[h - 2])
                            if tm == 4 and h >= 1 and st:
                                stats_s2(h - 1)
                            if tm == 5 and h >= 1 and st:
                                stats_s3(h - 1)
                            if tm == 6 and h >= 1 and st:
                                stats_s4(h - 1)
                            sq = None
                            if "nosq" not in KVAR:
                                sq = sp.tile([P, T], BF16, name="SQ", tag="SQ", bufs=3)
                                nc.vector.tensor_mul(sq[:], et[:], et[:])
                            for qh in range(2):
                                nc.tensor.matmul(
                                    pav[0:33, qh * 512 : (qh + 1) * 512],
                                    vt_sb[:, tm, 33 * h : 33 * h + 33],
                                    et[:, qh * 512 : (qh + 1) * 512],
                                    start=(tm == 0),
                                    stop=(tm == 7),
                                    skip_group_check=True,
                                )
                            if sq is not None:
                                for qh in range(2):
                                    nc.tensor.matmul(
                                        pav[64:65, qh * 512 : (qh + 1) * 512],
                                        ones_col[:],
                                        sq[:, qh * 512 : (qh + 1) * 512],
                                        start=(tm == 0),
                                        stop=(tm == 7),
                                        skip_group_check=True,
                                    )
                        # Z -> SBUF bf16 (DVE; keeps ACT exp-only)
                        if "nozrb" not in KVAR:
                            nc.vector.tensor_copy(zrb[:, h, :], pav[0:32, :])
                    if "notail" not in KVAR:
                        # drain: stats(7), tails 6 and 7
                        stats_s0(7)
                        zss[6] = tail_t0(6)
                        stats_s1(7)
                        tail_t1(6, zss[6])
                        stats_s2(7)
                        stats_s3(7)
                        stats_s4(7)
                        zss[7] = tail_t0(7)
                        tail_t1(7, zss[7])
                    else:
                        for oc in range(2):
                            nc.sync.dma_start(
                                yt_d[oc * P : (oc + 1) * P, :],
                                q_sb[oc][:].bitcast(F32),
                            )

            for _rep in range(reps):
                _one_rep()

    _split_excess_waits(nc)
    return nc


def _host_inputs(x, Wq, Wk, Wv, w_head, gamma, beta, Wp, bp):
    f = np.float32
    wpt = np.ascontiguousarray(np.asarray(Wp, f).T.reshape(8, HD, C).transpose(1, 0, 2))
    common = {
        "wqt": np.ascontiguousarray(np.asarray(Wq, f).T),
        "wkt": np.ascontiguousarray(np.asarray(Wk, f).T),
        "wvt": np.ascontiguousarray(np.asarray(Wv, f).T),
        "wst": np.ascontiguousarray(np.repeat(np.asarray(w_head, f), HD, axis=1).T),
        "wpt": wpt.astype(ml_dtypes.bfloat16),
        "wpct": np.ascontiguousarray(wpt.sum(1)),
        "bp": np.ascontiguousarray(np.asarray(bp, f).reshape(C, 1)),
        "gamma": np.ascontiguousarray(np.asarray(gamma, f).reshape(1, NH)),
        "beta": np.ascontiguousarray(np.asarray(beta, f).reshape(1, NH)),
    }
    xs = np.asarray(x, f).reshape(B, C, T)
    return [
        {"xf": np.ascontiguousarray(xs[b]), **common} for b in range(B)
    ]


_NC_CACHE = {}


def _get_nc(reps=1):
    if reps not in _NC_CACHE:
        _NC_CACHE[reps] = build_bass(reps=reps)
    return _NC_CACHE[reps]


def run(inputs, trace=False):
    nc = _get_nc()
    in_maps = _host_inputs(**inputs)
    res = run_bass_kernel_spmd(
        nc, in_maps, core_ids=list(range(N_CORES)), trace=trace
    )
    y = np.stack([res.results[b]["yt"] for b in range(B)], axis=0)
    return y.reshape(B, C, 32, 32).astype(np.float32), res


def _build_sharded(reps=1):
    """Replicate bass2jax.run_bass_via_pjrt but return a reusable callable
    (no donation) so device execution can be timed over many iterations."""
    import jax
    from jax.sharding import Mesh, PartitionSpec
    from jax.experimental.shard_map import shard_map
    from concourse import bass2jax

    nc = _get_nc(reps)
    bass2jax.install_neuronx_cc_hook()
    part_name = nc.partition_id_tensor.name if nc.partition_id_tensor else None
    in_names, out_names, out_avals = [], [], []
    for alloc in nc.m.functions[0].allocations:
        if not isinstance(alloc, mybir.MemoryLocationSet):
            continue
        name = alloc.memorylocations[0].name
        if alloc.kind == "ExternalInput":
            if name == part_name:
                continue
            in_names.append(name)
        elif alloc.kind == "ExternalOutput":
            out_names.append(name)
            out_avals.append(
                jax.core.ShapedArray(
                    tuple(alloc.tensor_shape), mybir.dt.np(alloc.dtype)
                )
            )
    n_params = len(in_names)
    all_in = in_names + out_names
    if part_name is not None:
        all_in = all_in + [part_name]

    def _body(*args):
        operands = list(args)
        if part_name is not None:
            operands.append(bass2jax.partition_id_tensor())
        outs = bass2jax._bass_exec_p.bind(
            *operands,
            out_avals=tuple(out_avals),
            in_names=tuple(all_in),
            out_names=tuple(out_names),
            lowering_input_output_aliases=(),
            sim_require_finite=True,
            sim_require_nnan=True,
            nc=nc,
        )
        return tuple(outs)

    devices = jax.devices()[:N_CORES]
    mesh = Mesh(np.asarray(devices), ("core",))
    nouts = len(out_names)
    sharded = jax.jit(
        shard_map(
            _body,
            mesh=mesh,
            in_specs=(PartitionSpec("core"),) * (n_params + nouts),
            out_specs=(PartitionSpec("core"),) * nouts,
            check_rep=False,
        ),
        keep_unused=True,
    )
    return sharded, mesh, in_names, out_names, out_avals


def timed_run(inputs, iters=20, reps=1):
    import time
    import jax
    from jax.sharding import NamedSharding, PartitionSpec

    sharded, mesh, in_names, out_names, out_avals = _build_sharded(reps)
    in_maps = _host_inputs(**inputs)
    sh = NamedSharding(mesh, PartitionSpec("core"))
    dev_in = [
        jax.device_put(
            np.concatenate([in_maps[c][n] for c in range(N_CORES)], axis=0), sh
        )
        for n in in_names
    ]
    dev_zero = [
        jax.device_put(
            np.zeros((N_CORES * a.shape[0], *a.shape[1:]), a.dtype), sh
        )
        for a in out_avals
    ]
    out = sharded(*dev_in, *dev_zero)
    jax.block_until_ready(out)
    # blocking per-call (includes full dispatch round trip)
    times = []
    for _ in range(max(3, iters // 4)):
        t0 = time.perf_counter()
        out = sharded(*dev_in, *dev_zero)
        jax.block_until_ready(out)
        times.append(time.perf_counter() - t0)
    # pipelined: submit all, block once -> amortizes host/axon dispatch
    t0 = time.perf_counter()
    outs = [sharded(*dev_in, *dev_zero) for _ in range(iters)]
    jax.block_until_ready(outs)
    pipelined = (time.perf_counter() - t0) / iters
    times.append(pipelined)
    print(f"pipelined per-call: {pipelined * 1e9:.0f} ns")
    y = np.asarray(outs[-1][out_names.index("yt")]).reshape(N_CORES, C, T)
    return y.reshape(B, C, 32, 32).astype(np.float32), times


def kernel(**inputs):
    y, _ = run(inputs, trace=False)
    return y


def numpy_check():
    """CoreSim single-core check against a numpy reference (core 0 data)."""
    from concourse.bass_interp import CoreSim

    rng = np.random.default_rng(0)
    x = rng.standard_normal((B, C, 32, 32), np.float32)
    Wq = (rng.standard_normal((C, C)) * 0.05).astype(np.float32)
    Wk = (rng.standard_normal((C, C)) * 0.05).astype(np.float32)
    Wv = (rng.standard_normal((C, C)) * 0.05).astype(np.float32)
    w_head = (rng.standard_normal((NH, NH)) * 0.3).astype(np.float32)
    gamma = rng.uniform(0.5, 1.5, NH).astype(np.float32)
    beta = (rng.standard_normal(NH) * 0.1).astype(np.float32)
    Wp = (rng.standard_normal((C, C)) * 0.05).astype(np.float32)
    bp = (rng.standard_normal(C) * 0.05).astype(np.float32)
    inputs = dict(
        x=x, Wq=Wq, Wk=Wk, Wv=Wv, w_head=w_head, gamma=gamma, beta=beta,
        Wp=Wp, bp=bp,
    )

    def ref_np(x, Wq, Wk, Wv, w_head, gamma, beta, Wp, bp):
        Bn, Cn, H, W = x.shape
        Tn = H * W
        hd = Cn // NH
        sc = float(hd) ** -0.5
        xf = x.reshape(Bn, Cn, Tn).astype(np.float64)
        q = np.einsum("oc,bct->bot", Wq, xf).reshape(Bn, NH, hd, Tn)
        k = np.einsum("oc,bct->bot", Wk, xf).reshape(Bn, NH, hd, Tn)
        v = np.einsum("oc,bct->bot", Wv, xf).reshape(Bn, NH, hd, Tn)
        s = np.einsum("bhdq,bhdt->bhqt", q, k) * sc
        s = np.einsum("hg,bgqt->bhqt", w_head.astype(np.float64), s)
        s = s - s.max(axis=-1, keepdims=True)
        e = np.exp(s)
        a = e / e.sum(-1, keepdims=True)
        mean = a.mean(axis=(2, 3), keepdims=True)
        var = a.var(axis=(2, 3), keepdims=True)
        g = gamma.astype(np.float64)[None, :, None, None]
        bt = beta.astype(np.float64)[None, :, None, None]
        a = (a - mean) / np.sqrt(var + EPS) * g + bt
        out = np.einsum("bhqt,bhdt->bhqd", a, v)
        y = out.reshape(Bn, Tn, Cn)
        y = np.einsum("btc,oc->bto", y, Wp.astype(np.float64)) + bp
        return y.transpose(0, 2, 1).reshape(Bn, Cn, H, W)

    expected = ref_np(**inputs)[0]  # core 0

    nc = _get_nc()
    in_maps = _host_inputs(**inputs)
    sim = CoreSim(nc, trace=False)
    for name, arr in in_maps[0].items():
        sim.tensor(name)[:] = arr
    sim.simulate(check_with_hw=False)
    got = np.array(sim.tensor("yt")).reshape(C, 32, 32)
    err = np.abs(got - expected) / (np.abs(expected) + 1e-3)
    print("max rel err (sim vs numpy f64):", err.max())
    print("mean rel err:", err.mean())
    return err.max()


if __name__ == "__main__":
    numpy_check()



# revision 8
# speedup vs baseline: 1.8291x; 1.1136x over previous
"""CMHSA Trainium2 kernel v2: per-head fused tail, direct-PSUM stats.

Per core (B=8 -> 8 cores data-parallel):
  q = Wq@xf, k = Wk@xf, v = Wv@xf            (C x T)
  s^T[h] = k^T @ (q . wst_h)  (K=256, w_head folded into q scaling via ACT
           Copy with per-partition scale)
  E = exp(sc * s^T) bf16 [t, q];  sq = E*E bf16 (DVE 2x)
  pav rows: 0:32 Z = v^T E (lhsT = [v^T | 1] -> row 32 = r), 64 rs2 = 1^T sq
  per-head stats read pav rows straight from PSUM (32-aligned partition
  shifts are legal): rinv = 1/r; ss = sum_q rs2*rinv^2; var = (ss-1)/T^2;
  alpha = gamma * rsqrt(var+eps)  (Newton rsqrt on DVE, no ACT table switch)
  s1 = alpha*rinv -> s1row (bf16, partition 0); biash = beta - alpha/T
  tail: gpsimd bcast s1 -> [32,T]; zs = zrb*s1 (bf16 2x);
    y[:, 128h:128h+128] = sum_j wpt_j^T @ zs[:, j::8] + u[:, h]; DMA out
  u[:, oc, h] = wtld[:, oc, h]*biash_h + bp   (wtld = wpct^T vs_dh, phase 1)
  Stats for head h are spread over head h+1's tm slots; tail of h runs
  during head h+2. The DVE/ACT/Pool queues stay off the PE critical path.
"""

import math
import os
import warnings

warnings.filterwarnings("ignore")

import numpy as np
import ml_dtypes

import concourse.bass as bass
import concourse.mybir as mybir
import concourse.tile as tile
from concourse import library_config
from concourse.bass_utils import run_bass_kernel_spmd

B, C, T, NH, HD, P = 8, 256, 1024, 8, 32, 128
EPS = 1e-5
SCALE = 1.0 / math.sqrt(HD)
F32 = mybir.dt.float32
F32R = mybir.dt.float32r
BF16 = mybir.dt.bfloat16
I32 = mybir.dt.int32
AF = mybir.ActivationFunctionType
ALU = mybir.AluOpType
N_CORES = 8
MAGIC = 0x5F3759DF
KVAR = set(os.environ.get("KVAR", "").split(","))


def _split_excess_waits(nc, max_waits=1):
    """This walrus build rejects >2 sem-waits on one instruction
    ("Too many sync wait commands" in setupSyncWait). Hoist excess waits
    onto same-engine NoOps inserted right before the offending instruction."""
    for f in nc.m.functions:
        for blk in f.blocks:
            insts = list(blk.instructions)
            out, changed = [], False
            for inst in insts:
                si = inst.sync_info
                waits = list(si.on_wait) if si and si.on_wait else []
                if len(waits) > max_waits:
                    extra, keep = waits[:-max_waits], waits[-max_waits:]
                    for w in extra:
                        nop = mybir.InstNoOp(
                            name=f"I-sw-{nc.next_id()}",
                            ins=[],
                            outs=[],
                            sync_info=mybir.SyncInfo(on_wait=[w], on_update=[]),
                            engine=inst.engine,
                        )
                        nc.register_instruction(nop)
                        out.append(nop)
                    inst.sync_info = mybir.SyncInfo(
                        on_wait=keep, on_update=list(si.on_update or [])
                    )
                    changed = True
                out.append(inst)
            if changed:
                blk.instructions = out


def build_bass(reps=1):
    nc = bass.Bass("TRN2", target_bir_lowering=False, debug=False)

    xf_d = nc.dram_tensor("xf", [C, T], F32R, kind="ExternalInput")
    wqt_d = nc.dram_tensor("wqt", [C, C], F32R, kind="ExternalInput")
    wkt_d = nc.dram_tensor("wkt", [C, C], F32R, kind="ExternalInput")
    wvt_d = nc.dram_tensor("wvt", [C, C], F32R, kind="ExternalInput")
    wst_d = nc.dram_tensor("wst", [C, NH], F32, kind="ExternalInput")
    wpt_d = nc.dram_tensor("wpt", [HD, 8, C], BF16, kind="ExternalInput")
    wpct_d = nc.dram_tensor("wpct", [HD, C], F32R, kind="ExternalInput")
    bp_d = nc.dram_tensor("bp", [C, 1], F32, kind="ExternalInput")
    gamma_d = nc.dram_tensor("gamma", [1, NH], F32, kind="ExternalInput")
    beta_d = nc.dram_tensor("beta", [1, NH], F32, kind="ExternalInput")
    yt_d = nc.dram_tensor("yt", [C, T], F32, kind="ExternalOutput")

    with tile.TileContext(nc) as tc:
        with (
            tc.tile_pool(name="w", bufs=1) as wp,
            tc.tile_pool(name="stream", bufs=2) as sp,
        ):
            # ---- persistent SBUF tensors ----
            xf_sb = [wp.tile([P, T], F32R, name=f"xf{i}", tag=f"xf{i}") for i in range(2)]
            wqt_sb = [wp.tile([P, C], F32R, name=f"wqt{i}", tag=f"wqt{i}") for i in range(2)]
            wkt_sb = [wp.tile([P, C], F32R, name=f"wkt{i}", tag=f"wkt{i}") for i in range(2)]
            wvt_sb = [wp.tile([P, C], F32R, name=f"wvt{i}", tag=f"wvt{i}") for i in range(2)]
            wst_sb = [wp.tile([P, NH], F32, name=f"wst{i}", tag=f"wst{i}") for i in range(2)]
            wpt_sb = wp.tile([HD, 8, C], BF16, name="wptj", tag="wptj")
            wpct_sb = wp.tile([HD, C], F32R, name="wpct", tag="wpct")
            bp_sb = [wp.tile([P, 1], F32, name=f"bp{i}", tag=f"bp{i}") for i in range(2)]
            gamma_sb = wp.tile([1, NH], F32, name="gamma", tag="gamma")
            beta_sb = wp.tile([1, NH], F32, name="beta", tag="beta")
            q_sb = [wp.tile([P, T], F32R, name=f"q{i}", tag=f"q{i}") for i in range(2)]
            k_sb = [wp.tile([P, T], F32R, name=f"k{i}", tag=f"k{i}") for i in range(2)]
            # v^T tiles: per t-chunk tm, 8 heads x (32 cols + ones col), bf16
            ET_DT = F32R if "etf32" in KVAR else BF16
            vt_sb = wp.tile([P, 8, NH * 33], ET_DT, name="vt", tag="vt")
            ones_col = wp.tile([P, 1], ET_DT, name="ones", tag="ones")
            ones_colf = wp.tile([P, 1], F32, name="onesf", tag="onesf")
            ones_row = wp.tile([1, P], BF16, name="onesr", tag="onesr")
            # per-head Z in bf16 [32, h, 1024]
            zrb = wp.tile([HD, NH, T], BF16, name="zrb", tag="zrb")
            s1row = wp.tile([1, NH * T], BF16, name="s1row", tag="s1row")
            ssr = wp.tile([1, NH], F32, name="ssr", tag="ssr")
            vpe = wp.tile([1, NH], F32, name="vpe", tag="vpe")
            itmp = wp.tile([1, NH], I32, name="itmp", tag="itmp")
            ftmp = wp.tile([1, NH], F32, name="ftmp", tag="ftmp")
            alpha = wp.tile([1, NH], F32, name="alpha", tag="alpha")
            biash = wp.tile([1, NH], BF16, name="biash", tag="biash")
            vs_row = wp.tile([1, C], F32R, name="vsrow", tag="vsrow")
            vs_dh = wp.tile([HD, NH], F32R, name="vsdh", tag="vsdh")
            wtld = wp.tile([P, 2, NH], F32, name="wtld", tag="wtld")
            u_sb = wp.tile([P, 2, NH], F32, name="u", tag="u")
            y_sb = [wp.tile([P, T], F32, name=f"y{i}", tag=f"y{i}") for i in range(2)]


            def _one_rep():
                # ---- phase 0: loads (q/k-critical first) ----
                for i in range(2):
                    nc.sync.dma_start(wqt_sb[i][:], wqt_d[i * P : (i + 1) * P, :])
                    nc.sync.dma_start(xf_sb[i][:], xf_d[i * P : (i + 1) * P, :])
                    nc.sync.dma_start(wkt_sb[i][:], wkt_d[i * P : (i + 1) * P, :])
                for i in range(2):
                    nc.sync.dma_start(wvt_sb[i][:], wvt_d[i * P : (i + 1) * P, :])
                    nc.sync.dma_start(wst_sb[i][:], wst_d[i * P : (i + 1) * P, :])
                    nc.sync.dma_start(bp_sb[i][:], bp_d[i * P : (i + 1) * P, :])
                nc.sync.dma_start(wpt_sb[:], wpt_d[:])
                nc.sync.dma_start(wpct_sb[:], wpct_d[:])
                nc.sync.dma_start(gamma_sb[:], gamma_d[:])
                nc.sync.dma_start(beta_sb[:], beta_d[:])
                nc.vector.memset(ones_row[:], 1.0)
                if ones_col.dtype == BF16:
                    nc.vector.memset(ones_col[:], 1.0)
                else:
                    nc.vector.memset(ones_colf[:], 1.0)
                    nc.vector.tensor_copy(ones_col[:], ones_colf[:])
                # ones columns inside vt (lhsT column 32 of each head block)
                for tm8 in range(8):
                    vt3 = vt_sb[:, tm8, :].rearrange("p (h e) -> p h e", e=33)
                    nc.vector.tensor_copy(
                        vt3[:, :, 32:33], ones_col[:, 0:1].broadcast_to((P, 8, 1))
                    )

                # ---- phase 1: projections ----
                with tc.tile_pool(name="psA", bufs=2, space=bass.MemorySpace.PSUM) as psA:
                    for wt, dst in ((wqt_sb, q_sb), (wkt_sb, k_sb)):
                        for co in range(2):
                            for tn in range(2):
                                pq = psA.tile([P, 512], F32, name="qk", tag="qk")
                                for kc in range(2):
                                    nc.tensor.matmul(
                                        pq[:],
                                        wt[kc][:, co * P : (co + 1) * P],
                                        xf_sb[kc][:, tn * 512 : (tn + 1) * 512],
                                        start=(kc == 0),
                                        stop=(kc == 1),
                                    )
                                nc.scalar.activation(
                                    dst[co][:, tn * 512 : (tn + 1) * 512], pq[:], AF.Copy
                                )
                    # v^T = xf^T @ Wv^T, written per t-chunk with head-stride 33
                    pvs = psA.tile([1, C], F32, name="vs", tag="vs", bufs=1)
                    for tm in range(8):
                        pv = psA.tile([P, C], F32, name="vt", tag="vt")
                        for kc in range(2):
                            nc.tensor.matmul(
                                pv[:],
                                xf_sb[kc][:, tm * P : (tm + 1) * P],
                                wvt_sb[kc][:],
                                start=(kc == 0),
                                stop=(kc == 1),
                            )
                        src = pv[:].rearrange("p (h d) -> p h d", h=NH)
                        dst3 = vt_sb[:, tm, :].rearrange("p (h e) -> p h e", e=33)
                        nc.scalar.activation(dst3[:, :, 0:32], src[:], AF.Copy)
                        nc.tensor.matmul(
                            pvs[:],
                            ones_col[:],
                            dst3[:, :, 0:32].rearrange("p h d -> p d h"),
                            start=(tm == 0),
                            stop=(tm == 7),
                            skip_group_check=True,
                        )
                    nc.scalar.activation(vs_row[:], pvs[:], AF.Copy)
                    # vs_dh [32, 8] <- vs_row [1, 256] partition-unflatten
                    nc.sync.dma_start(vs_dh[:], vs_row[:])
                    # wtld[:, oc, :] = wpct[:, oc]^T @ vs_dh   [128, 2, 8]
                    pw = psA.tile([P, 2, NH], F32, name="pw", tag="pw", bufs=1)
                    for oc in range(2):
                        nc.tensor.matmul(
                            pw[:, oc, :],
                            wpct_sb[:, oc * P : (oc + 1) * P],
                            vs_dh[:],
                            start=True,
                            stop=True,
                        )
                    nc.scalar.activation(wtld[:], pw[:], AF.Copy)

                # ---- phase 2: streaming attention + interleaved stats/tails
                qts = [None] * NH
                pavs = [None] * NH
                rinvs = [None] * NH

                def mk_qt(h):
                    # ACT Copy with per-partition scale: qt = q * wst[:, h]
                    qt = sp.tile([P, 2, T], F32R, name="qt", tag="qt")
                    for kc in range(2):
                        if "qtdve" in KVAR:
                            nc.vector.tensor_scalar_mul(
                                qt[:, kc, :], q_sb[kc][:], wst_sb[kc][:, h : h + 1]
                            )
                        else:
                            nc.scalar.activation(
                                qt[:, kc, :], q_sb[kc][:], AF.Copy,
                                scale=wst_sb[kc][:, h : h + 1],
                            )
                    qts[h] = qt

                def stats_s0(h):
                    rinv = sp.tile([1, T], F32, name="rinv", tag="rinv")
                    nc.vector.reciprocal(rinv[:], pavs[h][32:33, :])
                    rinvs[h] = rinv

                def stats_s1(h):
                    tt1 = sp.tile([1, T], F32, name="tt1", tag="tt1")
                    nc.vector.tensor_mul(tt1[:], pavs[h][64:65, :], rinvs[h][:])
                    tt2 = sp.tile([1, T], F32, name="tt2", tag="tt2")
                    nc.vector.scalar_tensor_tensor(
                        tt2[:], tt1[:], 1.0, rinvs[h][:],
                        op0=ALU.mult, op1=ALU.mult,
                        accum_out=ssr[0:1, h : h + 1],
                    )
                    nc.vector.tensor_scalar(
                        vpe[0:1, h : h + 1], ssr[0:1, h : h + 1],
                        1.0 / (T * T), EPS - 1.0 / (T * T),
                        op0=ALU.mult, op1=ALU.add,
                    )

                def stats_s2(h):
                    # alpha = gamma * rsqrt(vpe): Newton rsqrt on DVE
                    hs = slice(h, h + 1)
                    v_ = vpe[0:1, hs]
                    i_ = itmp[0:1, hs]
                    f_ = ftmp[0:1, hs]
                    y_ = i_.bitcast(F32)
                    nc.vector.tensor_scalar(
                        i_, v_.bitcast(I32), 1, None, op0=ALU.logical_shift_right
                    )
                    nc.vector.tensor_scalar(
                        i_, i_, -1, MAGIC, op0=ALU.mult, op1=ALU.add
                    )
                    for _ in range(2):
                        nc.vector.tensor_mul(f_, v_, y_)
                        nc.vector.tensor_mul(f_, f_, y_)
                        nc.vector.tensor_scalar(
                            f_, f_, -0.5, 1.5, op0=ALU.mult, op1=ALU.add
                        )
                        nc.vector.tensor_mul(i_.bitcast(F32), y_, f_)
                    nc.vector.tensor_mul(
                        alpha[0:1, hs], y_, gamma_sb[0:1, hs]
                    )
                    nc.vector.scalar_tensor_tensor(
                        biash[0:1, hs], alpha[0:1, hs], -1.0 / T, beta_sb[0:1, hs],
                        op0=ALU.mult, op1=ALU.add,
                    )

                def stats_s3(h):
                    # s1 row (bf16) at partition 0
                    nc.vector.tensor_scalar_mul(
                        s1row[0:1, h * T : (h + 1) * T], rinvs[h][:],
                        alpha[0:1, h : h + 1],
                    )

                def stats_s4(h):
                    # u[:, oc, h] = wtld * biash_h + bp; biash broadcast to
                    # 128 partitions via a tiny PE matmul (ones_row^T @ biash)
                    pbb = psS.tile([P, 512], F32, name="s", tag="s")
                    nc.tensor.matmul(
                        pbb[:, 0:1], ones_row[0:1, :], biash[0:1, h : h + 1],
                        start=True, stop=True,
                    )
                    for oc in range(2):
                        nc.vector.tensor_mul(
                            u_sb[:, oc, h : h + 1], wtld[:, oc, h : h + 1],
                            pbb[:, 0:1],
                        )
                        nc.vector.tensor_scalar_add(
                            u_sb[:, oc, h : h + 1], u_sb[:, oc, h : h + 1],
                            bp_sb[oc][:, 0:1],
                        )

                def tail_t0(h):
                    # broadcast s1 row to 32 partitions via ones_row matmul
                    # (psS "s"-tag tiles: no extra PSUM banks), then scale Z
                    zs = sp.tile([HD, T], BF16, name="zs", tag="zs")
                    for qh in range(2):
                        pb = psS.tile([P, 512], F32, name="s", tag="s")
                        nc.tensor.matmul(
                            pb[0:32, :],
                            ones_row[0:1, 0:32],
                            s1row[0:1, h * T + qh * 512 : h * T + (qh + 1) * 512],
                            start=True, stop=True,
                        )
                        nc.vector.tensor_mul(
                            zs[:, qh * 512 : (qh + 1) * 512],
                            zrb[:, h, qh * 512 : (qh + 1) * 512],
                            pb[0:32, :],
                        )
                    return zs

                def tail_t1(h, zs):
                    py = psB.tile([P, 2, P], F32, name="py", tag="py", bufs=1)
                    zs8 = zs[:].rearrange("p (m j) -> p m j", j=8)
                    for oc in range(2):
                        for j in range(8):
                            nc.tensor.matmul(
                                py[:, oc, :],
                                wpt_sb[:, j, oc * P : (oc + 1) * P],
                                zs8[:, :, j],
                                start=(j == 0),
                                stop=(j == 7),
                            )
                        nc.vector.tensor_scalar_add(
                            y_sb[oc][:, h * P : (h + 1) * P],
                            py[:, oc, :],
                            u_sb[:, oc, h : h + 1],
                        )
                        nc.sync.dma_start(
                            yt_d[oc * P : (oc + 1) * P, h * P : (h + 1) * P],
                            y_sb[oc][:, h * P : (h + 1) * P],
                        )

                psS_bufs = 2 if "ps1024" in KVAR else 3
                with (
                    tc.tile_pool(name="psS", bufs=psS_bufs, space=bass.MemorySpace.PSUM) as psS,
                    tc.tile_pool(name="psB", bufs=2, space=bass.MemorySpace.PSUM) as psB,
                ):
                    mk_qt(0)
                    zss = [None] * NH
                    for h in range(NH):
                        pav = psB.tile([65, T], F32, name="av", tag="av")
                        pavs[h] = pav
                        for tm in range(8):
                            et = sp.tile([P, T], ET_DT, name="E", tag="E", bufs=3)
                            if "ps1024" in KVAR:
                                ps = psS.tile([P, T], F32, name="s", tag="s")
                                for qh in range(2):
                                    for kc in range(2):
                                        nc.tensor.matmul(
                                            ps[:, qh * 512 : (qh + 1) * 512],
                                            k_sb[kc][:, tm * P : (tm + 1) * P],
                                            qts[h][:, kc, qh * 512 : (qh + 1) * 512],
                                            start=(kc == 0),
                                            stop=(kc == 1),
                                        )
                                nc.scalar.activation(et[:], ps[:], AF.Exp, scale=SCALE)
                            else:
                                for qh in range(2):
                                    ps = psS.tile([P, 512], F32, name="s", tag="s")
                                    for kc in range(2):
                                        nc.tensor.matmul(
                                            ps[:],
                                            k_sb[kc][:, tm * P : (tm + 1) * P],
                                            qts[h][:, kc, qh * 512 : (qh + 1) * 512],
                                            start=(kc == 0),
                                            stop=(kc == 1),
                                        )
                                    nc.scalar.activation(
                                        et[:, qh * 512 : (qh + 1) * 512], ps[:],
                                        AF.Exp, scale=SCALE,
                                    )
                            # interleaved work for previous heads
                            st = "notail" not in KVAR
                            if tm == 0 and h >= 1 and st:
                                stats_s0(h - 1)
                            if tm == 1 and h >= 2 and st:
                                zss[h - 2] = tail_t0(h - 2)
                            if tm == 2 and h + 1 < NH:
                                mk_qt(h + 1)
                            if tm == 2 and h >= 1 and st:
                                stats_s1(h - 1)
                            if tm == 3 and h >= 2 and st:
                                tail_t1(h - 2, zss[h - 2])
                            if tm == 4 and h >= 1 and st:
                                stats_s2(h - 1)
                            if tm == 5 and h >= 1 and st:
                                stats_s3(h - 1)
                            if tm == 6 and h >= 1 and st:
                                stats_s4(h - 1)
                            sq = None
                            if "nosq" not in KVAR:
                                sq = sp.tile([P, T], BF16, name="SQ", tag="SQ", bufs=3)
                                nc.vector.tensor_mul(sq[:], et[:], et[:])
                            for qh in range(2):
                                nc.tensor.matmul(
                                    pav[0:33, qh * 512 : (qh + 1) * 512],
                                    vt_sb[:, tm, 33 * h : 33 * h + 33],
                                    et[:, qh * 512 : (qh + 1) * 512],
                                    start=(tm == 0),
                                    stop=(tm == 7),
                                    skip_group_check=True,
                                )
                            if sq is not None:
                                for qh in range(2):
                                    nc.tensor.matmul(
                                        pav[64:65, qh * 512 : (qh + 1) * 512],
                                        ones_col[:],
                                        sq[:, qh * 512 : (qh + 1) * 512],
                                        start=(tm == 0),
                                        stop=(tm == 7),
                                        skip_group_check=True,
                                    )
                        # Z -> SBUF bf16 (DVE; keeps ACT exp-only)
                        if "nozrb" not in KVAR:
                            nc.vector.tensor_copy(zrb[:, h, :], pav[0:32, :])
                    if "notail" not in KVAR:
                        # drain: stats(7), tails 6 and 7
                        stats_s0(7)
                        zss[6] = tail_t0(6)
                        stats_s1(7)
                        tail_t1(6, zss[6])
                        stats_s2(7)
                        stats_s3(7)
                        stats_s4(7)
                        zss[7] = tail_t0(7)
                        tail_t1(7, zss[7])
                    else:
                        for oc in range(2):
                            nc.sync.dma_start(
                                yt_d[oc * P : (oc + 1) * P, :],
                                q_sb[oc][:].bitcast(F32),
                            )

            for _rep in range(reps):
                _one_rep()

    _split_excess_waits(nc)
    return nc


def _host_inputs(x, Wq, Wk, Wv, w_head, gamma, beta, Wp, bp):
    f = np.float32
    wpt = np.ascontiguousarray(np.asarray(Wp, f).T.reshape(8, HD, C).transpose(1, 0, 2))
    common = {
        "wqt": np.ascontiguousarray(np.asarray(Wq, f).T),
        "wkt": np.ascontiguousarray(np.asarray(Wk, f).T),
        "wvt": np.ascontiguousarray(np.asarray(Wv, f).T),
        "wst": np.ascontiguousarray(np.repeat(np.asarray(w_head, f), HD, axis=1).T),
        "wpt": wpt.astype(ml_dtypes.bfloat16),
        "wpct": np.ascontiguousarray(wpt.sum(1)),
        "bp": np.ascontiguousarray(np.asarray(bp, f).reshape(C, 1)),
        "gamma": np.ascontiguousarray(np.asarray(gamma, f).reshape(1, NH)),
        "beta": np.ascontiguousarray(np.asarray(beta, f).reshape(1, NH)),
    }
    xs = np.asarray(x, f).reshape(B, C, T)
    return [
        {"xf": np.ascontiguousarray(xs[b]), **common} for b in range(B)
    ]


_NC_CACHE = {}


def _get_nc(reps=1):
    if reps not in _NC_CACHE:
        _NC_CACHE[reps] = build_bass(reps=reps)
    return _NC_CACHE[reps]


def run(inputs, trace=False):
    nc = _get_nc()
    in_maps = _host_inputs(**inputs)
    res = run_bass_kernel_spmd(
        nc, in_maps, core_ids=list(range(N_CORES)), trace=trace
    )
    y = np.stack([res.results[b]["yt"] for b in range(B)], axis=0)
    return y.reshape(B, C, 32, 32).astype(np.float32), res


def _build_sharded(reps=1):
    """Replicate bass2jax.run_bass_via_pjrt but return a reusable callable
    (no donation) so device execution can be timed over many iterations."""
    import jax
    from jax.sharding import Mesh, PartitionSpec
    from jax.experimental.shard_map import shard_map
    from concourse import bass2jax

    nc = _get_nc(reps)
    bass2jax.install_neuronx_cc_hook()
    part_name = nc.partition_id_tensor.name if nc.partition_id_tensor else None
    in_names, out_names, out_avals = [], [], []
    for alloc in nc.m.functions[0].allocations:
        if not isinstance(alloc, mybir.MemoryLocationSet):
            continue
        name = alloc.memorylocations[0].name
        if alloc.kind == "ExternalInput":
            if name == part_name:
                continue
            in_names.append(name)
        elif alloc.kind == "ExternalOutput":
            out_names.append(name)
            out_avals.append(
                jax.core.ShapedArray(
                    tuple(alloc.tensor_shape), mybir.dt.np(alloc.dtype)
                )
            )
    n_params = len(in_names)
    all_in = in_names + out_names
    if part_name is not None:
        all_in = all_in + [part_name]

    def _body(*args):
        operands = list(args)
        if part_name is not None:
            operands.append(bass2jax.partition_id_tensor())
        outs = bass2jax._bass_exec_p.bind(
            *operands,
            out_avals=tuple(out_avals),
            in_names=tuple(all_in),
            out_names=tuple(out_names),
            lowering_input_output_aliases=(),
            sim_require_finite=True,
            sim_require_nnan=True,
            nc=nc,
        )
        return tuple(outs)

    devices = jax.devices()[:N_CORES]
    mesh = Mesh(np.asarray(devices), ("core",))
    nouts = len(out_names)
    sharded = jax.jit(
        shard_map(
            _body,
            mesh=mesh,
            in_specs=(PartitionSpec("core"),) * (n_params + nouts),
            out_specs=(PartitionSpec("core"),) * nouts,
            check_rep=False,
        ),
        keep_unused=True,
    )
    return sharded, mesh, in_names, out_names, out_avals


def timed_run(inputs, iters=20, reps=1):
    import time
    import jax
    from jax.sharding import NamedSharding, PartitionSpec

    sharded, mesh, in_names, out_names, out_avals = _build_sharded(reps)
    in_maps = _host_inputs(**inputs)
    sh = NamedSharding(mesh, PartitionSpec("core"))
    dev_in = [
        jax.device_put(
            np.concatenate([in_maps[c][n] for c in range(N_CORES)], axis=0), sh
        )
        for n in in_names
    ]
    dev_zero = [
        jax.device_put(
            np.zeros((N_CORES * a.shape[0], *a.shape[1:]), a.dtype), sh
        )
        for a in out_avals
    ]
    out = sharded(*dev_in, *dev_zero)
    jax.block_until_ready(out)
    # blocking per-call (includes full dispatch round trip)
    times = []
    for _ in range(max(3, iters // 4)):
        t0 = time.perf_counter()
        out = sharded(*dev_in, *dev_zero)
        jax.block_until_ready(out)
        times.append(time.perf_counter() - t0)
    # pipelined: submit all, block once -> amortizes host/axon dispatch
    t0 = time.perf_counter()
    outs = [sharded(*dev_in, *dev_zero) for _ in range(iters)]
    jax.block_until_ready(outs)
    pipelined = (time.perf_counter() - t0) / iters
    times.append(pipelined)
    print(f"pipelined per-call: {pipelined * 1e9:.0f} ns")
    y = np.asarray(outs[-1][out_names.index("yt")]).reshape(N_CORES, C, T)
    return y.reshape(B, C, 32, 32).astype(np.float32), times


def kernel(**inputs):
    y, _ = run(inputs, trace=False)
    return y


def numpy_check():
    """CoreSim single-core check against a numpy reference (core 0 data)."""
    from concourse.bass_interp import CoreSim

    rng = np.random.default_rng(0)
    x = rng.standard_normal((B, C, 32, 32), np.float32)
    Wq = (rng.standard_normal((C, C)) * 0.05).astype(np.float32)
    Wk = (rng.standard_normal((C, C)) * 0.05).astype(np.float32)
    Wv = (rng.standard_normal((C, C)) * 0.05).astype(np.float32)
    w_head = (rng.standard_normal((NH, NH)) * 0.3).astype(np.float32)
    gamma = rng.uniform(0.5, 1.5, NH).astype(np.float32)
    beta = (rng.standard_normal(NH) * 0.1).astype(np.float32)
    Wp = (rng.standard_normal((C, C)) * 0.05).astype(np.float32)
    bp = (rng.standard_normal(C) * 0.05).astype(np.float32)
    inputs = dict(
        x=x, Wq=Wq, Wk=Wk, Wv=Wv, w_head=w_head, gamma=gamma, beta=beta,
        Wp=Wp, bp=bp,
    )

    def ref_np(x, Wq, Wk, Wv, w_head, gamma, beta, Wp, bp):
        Bn, Cn, H, W = x.shape
        Tn = H * W
        hd = Cn // NH
        sc = float(hd) ** -0.5
        xf = x.reshape(Bn, Cn, Tn).astype(np.float64)
        q = np.einsum("oc,bct->bot", Wq, xf).reshape(Bn, NH, hd, Tn)
        k = np.einsum("oc,bct->bot", Wk, xf).reshape(Bn, NH, hd, Tn)
        v = np.einsum("oc,bct->bot", Wv, xf).reshape(Bn, NH, hd, Tn)
        s = np.einsum("bhdq,bhdt->bhqt", q, k) * sc
        s = np.einsum("hg,bgqt->bhqt", w_head.astype(np.float64), s)
        s = s - s.max(axis=-1, keepdims=True)
        e = np.exp(s)
        a = e / e.sum(-1, keepdims=True)
        mean = a.mean(axis=(2, 3), keepdims=True)
        var = a.var(axis=(2, 3), keepdims=True)
        g = gamma.astype(np.float64)[None, :, None, None]
        bt = beta.astype(np.float64)[None, :, None, None]
        a = (a - mean) / np.sqrt(var + EPS) * g + bt
        out = np.einsum("bhqt,bhdt->bhqd", a, v)
        y = out.reshape(Bn, Tn, Cn)
        y = np.einsum("btc,oc->bto", y, Wp.astype(np.float64)) + bp
        return y.transpose(0, 2, 1).reshape(Bn, Cn, H, W)

    expected = ref_np(**inputs)[0]  # core 0

    nc = _get_nc()
    in_maps = _host_inputs(**inputs)
    sim = CoreSim(nc, trace=False)
    for name, arr in in_maps[0].items():
        sim.tensor(name)[:] = arr
    sim.simulate(check_with_hw=False)
    got = np.array(sim.tensor("yt")).reshape(C, 32, 32)
    err = np.abs(got - expected) / (np.abs(expected) + 1e-3)
    print("max rel err (sim vs numpy f64):", err.max())
    print("mean rel err:", err.mean())
    return err.max()


if __name__ == "__main__":
    numpy_check()

